# revision 32
# baseline (speedup 1.0000x reference)
"""Trainium2 Bass kernel for nn_DenoiseNet (langevin point-cloud denoiser).

Strategy (8 NeuronCores, SPMD, zero inter-core communication):
  - Shard over B(2) x 4 contiguous N-chunks of 4096 points, each core padded
    with a 64-point halo on both sides (dependency cone grows 3 pts/step,
    4 steps -> 12 needed). Global-edge clipping handled exactly via per-core
    weight data (zeros on interior cores), so one program runs on all cores.
  - Feature-major fp16 layout [128 feat, (k, n) cols]. Sliding-window gather
    and scatter_add become free-dim shifted access patterns; the scatter
    k-sum rides matmul PSUM accumulation.
  - Residual adds are fused into scalar_tensor_tensor ops with a bias
    refold: h2' = max(ps1, -b1) + h0 equals (h0 + relu(ps1+b1)) - b1, and
    the -b1 shift is folded into the next layer's bias and the output
    constant. The per-column +bo constant is applied in the delta-update
    STT (with tiny edge-count corrections at the two global boundaries).
  - block1/block2 use k-paired [128,1024] PSUM tiles spanning two banks so
    one elementwise op drains two matmul outputs (strided pair APs via
    AP.rearrange); the scatter PSUM packs 4 rotating [3,512] slots into one
    bank at partition offsets 0/32/64/96.
  - Elementwise work is greedily load-balanced across ACT/DVE/GPSIMD using
    cost constants calibrated against the TimelineSim instruction model.
"""

import sys
import numpy as np

for _p in ("/opt/trn_rl_repo",):
    if _p not in sys.path:
        sys.path.insert(0, _p)

import concourse.bass as bass
import concourse.bacc as bacc
import concourse.tile as tile
from concourse import mybir
from concourse.bass_utils import run_bass_kernel_spmd

# ---- problem constants (hardcoded per harness contract) ----
B, N, D = 2, 16384, 3
F = 128
K = 4
OFF = [-2, -1, 0, 1]
STEPS, S0, DECAY = 4, 0.2, 0.95
CHUNK, HALO, GW = 4096, 64, 2
NP = CHUNK + 2 * HALO          # 4224 local points
NB = NP + 2 * GW               # 4228 buffer cols (with guards)
R4 = K * NP                    # 16896 (k,n) columns
N_CORES = 8

f32 = mybir.dt.float32
f16 = mybir.dt.float16
AF = mybir.ActivationFunctionType
ALU = mybir.AluOpType

_CH512 = [(c * 512, min(512, NP - c * 512)) for c in range((NP + 511) // 512)]
_CHNB = [(c * 512, min(512, NB - c * 512)) for c in range((NB + 511) // 512)]


def build_program(reps=1, loop_n=0, zb=True):
    """Build the SPMD Bass/Tile program. Returns compiled Bacc module."""
    nc = bacc.Bacc("TRN2", target_bir_lowering=False, debug=False)

    def inp(name, shape, dt):
        return nc.dram_tensor(name, list(shape), dt, kind="ExternalInput").ap()

    d_pclT = inp("pclT", (4, NB), f16)
    d_delta0 = inp("delta0", (4, NB), f16)
    d_Wf1 = inp("Wf1", (3, F), f16)
    d_bf1 = inp("bf1", (F, 1), f32)
    d_WfW = inp("WfW", (F, F), f16)
    d_bg = inp("bg", (F, 1), f32)
    d_W0g = inp("W0g", (3, F), f16)
    d_W0gn = inp("W0gn", (3, F), f16)
    d_I128 = inp("I128", (F, F), f16)
    d_Wb1 = inp("Wb1", (F, F), f16)
    d_Wb2 = inp("Wb2", (F, F), f16)
    d_pb1 = inp("pb1", (F, 1), f32)     # +b1
    d_nb1 = inp("nb1", (F, 1), f32)     # -b1
    d_pb2 = inp("pb2", (F, 1), f32)     # +b2eff
    d_nb2 = inp("nb2", (F, 1), f32)     # -b2eff
    d_WoS = inp("WoS", (F, 3 * STEPS), f16)
    d_bo4 = inp("bo4", (4, STEPS), f32)        # 4*bo_eff per step
    d_cboL = inp("cboL", (4, 2 * STEPS), f32)  # edge count corrections
    d_cboR = inp("cboR", (4, 2 * STEPS), f32)
    d_eL = inp("eL", (F, 3 * STEPS), f16)
    d_eLn = inp("eLn", (F, 3 * STEPS), f16)
    d_eR = inp("eR", (F, 3 * STEPS), f16)
    d_eRn = inp("eRn", (F, 3 * STEPS), f16)
    d_flagL = inp("flagL", (4, 1), f32)
    d_flagR = inp("flagR", (4, 1), f32)
    d_out = nc.dram_tensor("outT", [4, CHUNK], f16, kind="ExternalOutput").ap()

    from contextlib import ExitStack
    with tile.TileContext(nc) as tc, ExitStack() as ctx:
        cpool = ctx.enter_context(tc.tile_pool(name="const", bufs=1))
        hpool = ctx.enter_context(tc.tile_pool(name="h", bufs=4))
        tp2 = ctx.enter_context(tc.tile_pool(name="t2", bufs=5))
        tpool = ctx.enter_context(tc.tile_pool(name="tiny", bufs=2))
        psp = ctx.enter_context(tc.tile_pool(name="ps", bufs=3, space="PSUM"))
        psbc = ctx.enter_context(tc.tile_pool(name="psBC", bufs=2, space="PSUM"))
        pssc = ctx.enter_context(tc.tile_pool(name="psS", bufs=1, space="PSUM"))
        h0pool = ctx.enter_context(tc.tile_pool(name="h0p", bufs=14))
        h2pool = ctx.enter_context(tc.tile_pool(name="h2p", bufs=7))

        def load(dram, shape, dt, tag):
            t = cpool.tile(list(shape), dt, tag=tag)
            nc.sync.dma_start(t[:], dram[:])
            return t

        pclT = load(d_pclT, (4, NB), f16, "pclT")
        delta_a = load(d_delta0, (4, NB), f16, "delta_a")
        delta_b = load(d_delta0, (4, NB), f16, "delta_b")
        Wf1 = load(d_Wf1, (3, F), f16, "Wf1")
        bf1 = load(d_bf1, (F, 1), f32, "bf1")
        WfW = load(d_WfW, (F, F), f16, "WfW")
        bg = load(d_bg, (F, 1), f32, "bg")
        W0g = load(d_W0g, (3, F), f16, "W0g")
        W0gn = load(d_W0gn, (3, F), f16, "W0gn")
        I128 = load(d_I128, (F, F), f16, "I128")
        Wb1 = load(d_Wb1, (F, F), f16, "Wb1")
        Wb2 = load(d_Wb2, (F, F), f16, "Wb2")
        pb1 = load(d_pb1, (F, 1), f32, "pb1")
        nb1 = load(d_nb1, (F, 1), f32, "nb1")
        pb2 = load(d_pb2, (F, 1), f32, "pb2")
        nb2 = load(d_nb2, (F, 1), f32, "nb2")
        WoS = load(d_WoS, (F, 3 * STEPS), f16, "WoS")
        bo4 = load(d_bo4, (4, STEPS), f32, "bo4")
        cboL = load(d_cboL, (4, 2 * STEPS), f32, "cboL")
        cboR = load(d_cboR, (4, 2 * STEPS), f32, "cboR")
        eL = load(d_eL, (F, 3 * STEPS), f16, "eL")
        eLn = load(d_eLn, (F, 3 * STEPS), f16, "eLn")
        eR = load(d_eR, (F, 3 * STEPS), f16, "eR")
        eRn = load(d_eRn, (F, 3 * STEPS), f16, "eRn")
        flagL = load(d_flagL, (4, 1), f32, "flagL")
        flagR = load(d_flagR, (4, 1), f32, "flagR")

        Gk = cpool.tile([F, R4], f16, tag="Gk")
        h3_a = cpool.tile([F, R4], f16, tag="h3_a")
        h3_b = cpool.tile([F, R4], f16, tag="h3_b")
        A0e = cpool.tile([F, NB], f16, tag="A0e")
        G0 = cpool.tile([F, NP], f16, tag="G0")

        # scatter PSUM: one bank holding 3 rotating [3,512] slots at
        # partition offsets 0/32/64 (96 is not a legal base partition)
        ps_sc = pssc.tile([F, 512], f32, tag="S")
        sc_ctr = [0]

        def pairv(t, fd):
            # [P, 1024] tile -> strided pair view [P, 2, fd] (halves at
            # byte offsets 0 and 512 cols)
            return t[:].rearrange("p (h n) -> p h n", h=2)[:, :, 0:fd]

        # ---- greedy tri-engine balancer for elementwise work ----
        # constants calibrated against the TimelineSim instruction model
        load_ns = {"ACT": 0.0, "DVE": 0.0, "POOL": 0.0, "DMA": 0.0}

        def c_act(fd):
            return fd * 0.833 + 190

        def c_dve_ps(fd):
            return fd * 1.042 + 125

        def c_dve_stt_sb(fd):
            return fd * 1.042 + 60

        def c_dve_tt(fd):
            return fd * 0.521 + 60

        def c_pool_tt(fd):
            return fd * 1.984 + 120

        def pick(options):
            # options: list of (fn, [(eng, cost), ...])
            best = None
            for fn, usages in options:
                new = dict(load_ns)
                for e, c in usages:
                    new[e] += c
                key = (max(new.values()), sum(c for _, c in usages))
                if best is None or key < best[0]:
                    best = (key, fn, usages)
            for e, c in best[2]:
                load_ns[e] += c
            best[1]()

        def charge(eng, cost):
            load_ns[eng] += cost

        def relu_op(dst, src, fd, bias=None):
            # psum -> sbuf relu, optional per-partition bias. GPSIMD cannot
            # read PSUM, but a DMA drain (psum -> sbuf staging) lets Pool do
            # the relu as TT-max against zeros (bias-free case only).
            def on_act():
                nc.scalar.activation(dst, src, AF.Relu,
                                     bias=(bias[:, :] if bias is not None else 0.0))
            def on_dve():
                if bias is not None:
                    nc.vector.tensor_scalar(dst, src, bias[:, :], 0.0, ALU.add, ALU.max)
                else:
                    nc.vector.tensor_scalar_max(dst, src, 0.0)
            pick([(on_act, [("ACT", c_act(fd))]),
                  (on_dve, [("DVE", c_dve_ps(fd))])])

        def copy_op(dst, src, fd):
            def on_act():
                nc.scalar.activation(dst, src, AF.Copy)
            def on_dve():
                nc.vector.tensor_copy(dst, src)
            pick([(on_act, [("ACT", c_act(fd))]),
                  (on_dve, [("DVE", c_dve_ps(fd))])])

        def fused_resid_pair(dst_pair, ps_pair, hin_pair, fd, pb, nb):
            # dst = max(ps, -b) + hin over a k-pair (== hin + relu(ps+b) - b)
            # GPSIMD cannot read PSUM and only supports TensorTensor-class
            # ops: its 2-op path (ACT relu -> sbuf, Pool TT add) is exact
            # only when b == 0, so it is only emitted in zero-bias programs.
            fd2 = 2 * fd
            def on_dve():
                nc.vector.scalar_tensor_tensor(dst_pair, ps_pair, nb[:, :],
                                               hin_pair, ALU.max, ALU.add)
            def on_2op():
                t = tp2.tile([F, 1024], f16, tag="t2")
                tv = pairv(t, fd)
                nc.scalar.activation(tv, ps_pair, AF.Relu, bias=pb[:, :])
                if zb:
                    nc.vector.tensor_add(dst_pair, tv, hin_pair)
                else:
                    nc.vector.scalar_tensor_tensor(dst_pair, tv, nb[:, :],
                                                   hin_pair, ALU.add, ALU.add)
            def on_2op_pool():
                t = tp2.tile([F, 1024], f16, tag="t2")
                tv = pairv(t, fd)
                nc.scalar.activation(tv, ps_pair, AF.Relu)
                nc.gpsimd.tensor_add(dst_pair, tv, hin_pair)
            def on_dma():
                # relu straight into dst, then the residual add rides the
                # software-DGE accumulate path (idle DMA rings)
                nc.scalar.activation(dst_pair, ps_pair, AF.Relu)
                nc.gpsimd.dma_start(dst_pair, hin_pair, accum_op=ALU.add)
            opts = [(on_dve, [("DVE", c_dve_ps(fd2))]),
                    (on_2op, [("ACT", c_act(fd2)),
                              ("DVE", c_dve_tt(fd2) if zb else c_dve_stt_sb(fd2))])]
            if zb:
                opts.append(
                    (on_2op_pool, [("ACT", c_act(fd2)), ("POOL", c_pool_tt(fd2))]))
            pick(opts)

        def delta_update(dst, ps, din, fd, step):
            # dst = ps + 4*bo_eff[step] + din
            sc = bo4[0:3, step:step + 1]
            def on_dve():
                nc.vector.scalar_tensor_tensor(dst, ps, sc, din, ALU.add, ALU.add)
            def on_2op():
                t = hpool.tile([4, 512], f16, tag="t4")
                nc.scalar.activation(t[0:3, :fd], ps, AF.Identity, bias=sc)
                nc.vector.tensor_add(dst, t[0:3, :fd], din)
            def on_2op_pool():
                t = hpool.tile([4, 512], f16, tag="t4")
                nc.scalar.activation(t[0:3, :fd], ps, AF.Identity, bias=sc)
                nc.gpsimd.tensor_add(dst, t[0:3, :fd], din)
            def on_dma():
                nc.scalar.activation(dst, ps, AF.Identity, bias=sc)
                nc.gpsimd.dma_start(dst, din, accum_op=ALU.add)
            pick([(on_dve, [("DVE", c_dve_ps(fd))]),
                  (on_2op, [("ACT", c_act(fd)), ("DVE", c_dve_tt(fd))]),
                  (on_2op_pool, [("ACT", c_act(fd)), ("POOL", c_pool_tt(fd))])])

        # one column at the k=2/k=3 boundary is read (as cone garbage) by the
        # interleaved scatter before any tile writes it on step 0
        nc.vector.memset(h3_a[:, 3 * NP - 1:3 * NP], 0.0)
        nc.vector.memset(h3_b[:, 3 * NP - 1:3 * NP], 0.0)

        # ---------------- preamble: A0e, G0, Gk ----------------
        for ci, (c0, fd) in enumerate(_CHNB):
            ps = psp.tile([F, 512], f32, tag="ps")
            nc.tensor.matmul(ps[:, :fd], W0g[:, :], pclT[0:3, c0:c0 + fd],
                             start=True, stop=True)
            copy_op(A0e[:, c0:c0 + fd], ps[:, :fd], fd)
        for ci, (c0, fd) in enumerate(_CH512):
            ps = psp.tile([F, 512], f32, tag="ps")
            nc.tensor.matmul(ps[:, :fd], Wf1[:, :], pclT[0:3, GW + c0:GW + c0 + fd],
                             start=True, stop=True)
            hf = hpool.tile([F, 512], f16, tag="h0")
            nc.scalar.activation(hf[:, :fd], ps[:, :fd], AF.Relu, bias=bf1[:, :])
            ps2 = psp.tile([F, 512], f32, tag="ps")
            nc.tensor.matmul(ps2[:, :fd], WfW[:, :], hf[:, :fd], start=True, stop=False)
            nc.tensor.matmul(ps2[:, :fd], W0gn[:, :], pclT[0:3, GW + c0:GW + c0 + fd],
                             start=False, stop=True)
            nc.scalar.activation(G0[:, c0:c0 + fd], ps2[:, :fd], AF.Identity, bias=bg[:, :])
        for k in range(K):
            for c0, fd in _CH512:
                def do_add(k=k, c0=c0, fd=fd):
                    nc.vector.tensor_add(
                        Gk[:, k * NP + c0:k * NP + c0 + fd], G0[:, c0:c0 + fd],
                        A0e[:, GW + OFF[k] + c0:GW + OFF[k] + c0 + fd])
                def do_add_p(k=k, c0=c0, fd=fd):
                    nc.gpsimd.tensor_add(
                        Gk[:, k * NP + c0:k * NP + c0 + fd], G0[:, c0:c0 + fd],
                        A0e[:, GW + OFF[k] + c0:GW + OFF[k] + c0 + fd])
                pick([(do_add, [("DVE", c_dve_tt(fd))]),
                      (do_add_p, [("POOL", c_pool_tt(fd))])])

        # ---------------- langevin steps ----------------
        def emit_rep(final_rep):
            for step in range(STEPS):
                d_in = delta_a if step % 2 == 0 else delta_b
                d_out_t = delta_b if step % 2 == 0 else delta_a
                h3 = h3_a if step % 2 == 0 else h3_b
                h3kv = h3[:].rearrange("p (k n) -> p k n", k=K)
                final = (step == STEPS - 1) and final_rep
                s3 = slice(3 * step, 3 * step + 3)

                def emit_passA(cb):
                    c0, fd = _CH512[cb]
                    for kk in (0, 2):
                        h0p = h0pool.tile([F, 1024], f16, tag="h0")
                        for j in range(2):
                            k = kk + j
                            ps = psp.tile([F, 512], f32, tag="ps")
                            nc.tensor.matmul(
                                ps[:, :fd], W0g[:, :],
                                d_in[0:3, GW + OFF[k] + c0:GW + OFF[k] + c0 + fd],
                                start=True, stop=False)
                            nc.tensor.matmul(ps[:, :fd], I128[:, :],
                                             Gk[:, k * NP + c0:k * NP + c0 + fd],
                                             start=False, stop=True)
                            relu_op(h0p[:, 512 * j:512 * j + fd], ps[:, :fd], fd)
                        h0s[(kk, cb)] = h0p

                def emit_passB(cb):
                    c0, fd = _CH512[cb]
                    for kk in (0, 2):
                        h0p = h0s[(kk, cb)]
                        ps = psbc.tile([F, 1024], f32, tag="bc")
                        for j in range(2):
                            nc.tensor.matmul(ps[:, 512 * j:512 * j + fd], Wb1[:, :],
                                             h0p[:, 512 * j:512 * j + fd],
                                             start=True, stop=True)
                        h2p = h2pool.tile([F, 1024], f16, tag="h2")
                        fused_resid_pair(pairv(h2p, fd), pairv(ps, fd),
                                         pairv(h0p, fd), fd, pb1, nb1)
                        h2s[(kk, cb)] = h2p

                def emit_passC(cb):
                    c0, fd = _CH512[cb]
                    for kk in (0, 2):
                        h2p = h2s[(kk, cb)]
                        ps = psbc.tile([F, 1024], f32, tag="bc")
                        for j in range(2):
                            nc.tensor.matmul(ps[:, 512 * j:512 * j + fd], Wb2[:, :],
                                             h2p[:, 512 * j:512 * j + fd],
                                             start=True, stop=True)
                        fused_resid_pair(h3kv[:, kk:kk + 2, c0:c0 + fd],
                                         pairv(ps, fd), pairv(h2p, fd), fd, pb2, nb2)

                def mirror_fix(flag, src_l, dst_l0, w):
                    # mirror guards at global edges (flag=0 -> no-op on
                    # interior); handles w contiguous dst columns in one shot
                    # with a broadcast source
                    dst = d_out_t[0:3, GW + dst_l0:GW + dst_l0 + w]
                    src = d_out_t[0:3, GW + src_l:GW + src_l + 1].broadcast_to([3, w])
                    t = tpool.tile([4, 2], f16, tag="mir")
                    nc.vector.tensor_sub(t[0:3, :w], src, dst)
                    nc.vector.tensor_scalar_mul(t[0:3, :w], t[0:3, :w], flag[0:3, :])
                    nc.vector.tensor_add(dst, dst, t[0:3, :w])

                def emit_scatter(cb):
                    c0, fd = _CH512[cb]
                    p0 = 32 * (sc_ctr[0] % 3)
                    sc_ctr[0] += 1
                    ps = ps_sc[p0:p0 + 4, :]
                    mms = []
                    for k in range(K):
                        st = k * NP + c0 - OFF[k]
                        mms.append((ps[0:3, :fd], WoS[:, s3], h3[:, st:st + fd]))
                    if cb == 0:
                        pcol = ps[0:3, HALO:HALO + 1]
                        for col in (HALO, HALO + 1, NP + HALO):
                            mms.append((pcol, eL[:, s3], h3[:, col:col + 1]))
                        mms.append((pcol, eLn[:, s3],
                                    h3[:, 3 * NP + HALO - 1:3 * NP + HALO]))
                    if cb == len(_CH512) - 1:
                        lN = HALO + CHUNK - 1
                        pN = ps[0:3, lN - c0:lN - c0 + 1]
                        mms.append((pN, eR[:, s3], h3[:, 3 * NP + lN:3 * NP + lN + 1]))
                        for col in (lN + 2, NP + lN + 1):
                            mms.append((pN, eRn[:, s3], h3[:, col:col + 1]))
                        mms.append((ps[0:3, lN - 1 - c0:lN - c0], eRn[:, s3],
                                    h3[:, lN + 1:lN + 2]))
                    for i, (o, w, m) in enumerate(mms):
                        nc.tensor.matmul(o, w, m, start=(i == 0),
                                         stop=(i == len(mms) - 1))
                    delta_update(d_out_t[0:3, GW + c0:GW + c0 + fd], ps[0:3, :fd],
                                 d_in[0:3, GW + c0:GW + c0 + fd], fd, step)
                    if cb == 0:
                        nc.vector.tensor_add(
                            d_out_t[0:3, GW + HALO:GW + HALO + 2],
                            d_out_t[0:3, GW + HALO:GW + HALO + 2],
                            cboL[0:3, 2 * step:2 * step + 2])
                        charge("DVE", 80)
                    if cb == len(_CH512) - 1:
                        r0 = HALO + CHUNK - 2
                        nc.vector.tensor_add(
                            d_out_t[0:3, GW + r0:GW + r0 + 2],
                            d_out_t[0:3, GW + r0:GW + r0 + 2],
                            cboR[0:3, 2 * step:2 * step + 2])
                        charge("DVE", 80)

                h0s = {}
                h2s = {}
                nblk = len(_CH512)
                for cb in range(nblk + 5):
                    if cb < nblk:
                        emit_passA(cb)
                    if 0 <= cb - 2 < nblk:
                        emit_passB(cb - 2)
                    if 0 <= cb - 3 < nblk:
                        emit_passC(cb - 3)
                    if 0 <= cb - 5 < nblk:
                        emit_scatter(cb - 5)

                if final:
                    nc.sync.dma_start(
                        d_out[:, :], d_out_t[0:4, GW + HALO:GW + HALO + CHUNK])
                else:
                    mirror_fix(flagL, HALO, HALO - 2, 2)
                    mirror_fix(flagR, HALO + CHUNK - 1, HALO + CHUNK, 1)

        if loop_n:
            with tc.For_i(0, loop_n, 1):
                emit_rep(False)
            emit_rep(True)
        else:
            for rep in range(reps):
                emit_rep(rep == reps - 1)

    nc.compile()
    return nc


def host_prep(inputs):
    """Slice/transpose/pad inputs per core; build weight-variant constants."""
    pcl = np.asarray(inputs["pcl_noisy"], np.float32)
    Wf1 = np.asarray(inputs["Wf1"], np.float32)
    bf1 = np.asarray(inputs["bf1"], np.float32)
    Wf2 = np.asarray(inputs["Wf2"], np.float32)
    bf2 = np.asarray(inputs["bf2"], np.float32)
    W0 = np.asarray(inputs["W0"], np.float32)
    b0 = np.asarray(inputs["b0"], np.float32)
    Wb = np.asarray(inputs["Wb"], np.float32)
    bb = np.asarray(inputs["bb"], np.float32)
    Wo = np.asarray(inputs["Wo"], np.float32)
    bo = np.asarray(inputs["bo"], np.float32)

    W0g = W0[:3]
    WfW = Wf2 @ W0[3:]
    bg = bf2 @ W0[3:] + b0
    offs = np.arange(-(K - 1) // 2, (K - 1) // 2 + 1)
    nbr = np.clip(np.arange(N)[:, None] + offs, 0, N - 1).reshape(-1)
    counts = np.bincount(nbr, minlength=N).astype(np.float32)

    b1 = bb[0]
    b2eff = bb[1] + Wb[1].T @ b1
    bsum = b1 + b2eff
    svals = [S0 * DECAY ** i for i in range(STEPS)]
    WoS = np.concatenate([s * Wo for s in svals], axis=1)          # [128, 12]
    bo_eff = np.stack([s * (Wo.T @ bsum + bo) for s in svals], 1)  # [3, STEPS]
    bo4 = np.zeros((4, STEPS), np.float32)
    bo4[0:3] = 4.0 * bo_eff

    hf = np.float16
    shared = {
        "Wf1": Wf1.astype(hf), "bf1": bf1.reshape(F, 1),
        "WfW": WfW.astype(hf), "bg": bg.reshape(F, 1),
        "W0g": W0g.astype(hf), "W0gn": (-W0g).astype(hf),
        "I128": np.eye(F, dtype=np.float32).astype(hf),
        "Wb1": Wb[0].astype(hf), "Wb2": Wb[1].astype(hf),
        "pb1": b1.reshape(F, 1).astype(np.float32),
        "nb1": (-b1).reshape(F, 1).astype(np.float32),
        "pb2": b2eff.reshape(F, 1).astype(np.float32),
        "nb2": (-b2eff).reshape(F, 1).astype(np.float32),
        "WoS": WoS.astype(hf),
        "bo4": bo4,
    }
    zeros_e = np.zeros((F, 3 * STEPS), np.float16)
    zeros_c = np.zeros((4, 2 * STEPS), np.float32)
    # edge-count corrections: counts - 4 is nonzero only at global cols
    # {0, 1} (left) and {N-2, N-1} (right)
    cboL = np.zeros((4, 2 * STEPS), np.float32)
    cboR = np.zeros((4, 2 * STEPS), np.float32)
    for s in range(STEPS):
        cboL[0:3, 2 * s:2 * s + 2] = (counts[0:2] - 4.0) * bo_eff[:, s:s + 1]
        cboR[0:3, 2 * s:2 * s + 2] = (counts[N - 2:N] - 4.0) * bo_eff[:, s:s + 1]

    in_maps = []
    for core in range(N_CORES):
        b, ch = core // 4, core % 4
        g0 = ch * CHUNK - HALO
        idx = np.clip(np.arange(g0 - GW, g0 + NP + GW), 0, N - 1)
        pclT = np.empty((4, NB), np.float16)
        pclT[0:3] = pcl[b, idx].T.astype(np.float16)
        pclT[3] = 0.0
        delta0 = np.zeros((4, NB), np.float16)
        isL, isR = ch == 0, ch == 3
        m = dict(shared)
        m["pclT"] = pclT
        m["delta0"] = delta0
        m["eL"] = (WoS.astype(hf) if isL else zeros_e)
        m["eLn"] = ((-WoS).astype(hf) if isL else zeros_e)
        m["eR"] = (WoS.astype(hf) if isR else zeros_e)
        m["eRn"] = ((-WoS).astype(hf) if isR else zeros_e)
        m["cboL"] = (cboL if isL else zeros_c)
        m["cboR"] = (cboR if isR else zeros_c)
        m["flagL"] = np.full((4, 1), 1.0 if isL else 0.0, np.float32)
        m["flagR"] = np.full((4, 1), 1.0 if isR else 0.0, np.float32)
        in_maps.append(m)
    return in_maps


_CACHED = {}


def _get_program(reps=1, zb=True):
    key = (reps, zb)
    if key not in _CACHED:
        _CACHED[key] = build_program(reps, zb=zb)
    return _CACHED[key]


def kernel(**inputs):
    zb = all(
        not np.any(np.asarray(inputs[k]))
        for k in ("bf1", "bf2", "b0", "bb", "bo"))
    nc = _get_program(1, zb)
    in_maps = host_prep(inputs)
    res = run_bass_kernel_spmd(nc, in_maps, list(range(N_CORES)))
    pcl = np.asarray(inputs["pcl_noisy"], np.float32)
    out = np.empty((B, N, D), np.float32)
    for core in range(N_CORES):
        b, ch = core // 4, core % 4
        sl = slice(ch * CHUNK, (ch + 1) * CHUNK)
        out[b, sl] = pcl[b, sl] + res.results[core]["outT"][0:3].T.astype(np.float32)
    return out


# revision 35
# speedup vs baseline: 1.0339x; 1.0339x over previous
"""Trainium2 Bass kernel for nn_DenoiseNet (langevin point-cloud denoiser).

Strategy (8 NeuronCores, SPMD, zero inter-core communication):
  - Shard over B(2) x 4 contiguous N-chunks of 4096 points, each core padded
    with a 64-point halo on both sides (dependency cone grows 3 pts/step,
    4 steps -> 12 needed). Global-edge clipping handled exactly via per-core
    weight data (zeros on interior cores), so one program runs on all cores.
  - Feature-major fp16 layout [128 feat, (k, n) cols]. Sliding-window gather
    and scatter_add become free-dim shifted access patterns; the scatter
    k-sum rides matmul PSUM accumulation.
  - Residual adds are fused into scalar_tensor_tensor ops with a bias
    refold: h2' = max(ps1, -b1) + h0 equals (h0 + relu(ps1+b1)) - b1, and
    the -b1 shift is folded into the next layer's bias and the output
    constant. The per-column +bo constant is applied in the delta-update
    STT (with tiny edge-count corrections at the two global boundaries).
  - block1/block2 use k-paired [128,1024] PSUM tiles spanning two banks so
    one elementwise op drains two matmul outputs (strided pair APs via
    AP.rearrange); the scatter PSUM packs 4 rotating [3,512] slots into one
    bank at partition offsets 0/32/64/96.
  - Elementwise work is greedily load-balanced across ACT/DVE/GPSIMD using
    cost constants calibrated against the TimelineSim instruction model.
"""

import sys
import numpy as np

for _p in ("/opt/trn_rl_repo",):
    if _p not in sys.path:
        sys.path.insert(0, _p)

import concourse.bass as bass
import concourse.bacc as bacc
import concourse.tile as tile
from concourse import mybir
from concourse.bass_utils import run_bass_kernel_spmd

# ---- problem constants (hardcoded per harness contract) ----
B, N, D = 2, 16384, 3
F = 128
K = 4
OFF = [-2, -1, 0, 1]
STEPS, S0, DECAY = 4, 0.2, 0.95
CHUNK, HALO, GW = 4096, 64, 2
NP = CHUNK + 2 * HALO          # 4224 local points
NB = NP + 2 * GW               # 4228 buffer cols (with guards)
R4 = K * NP                    # 16896 (k,n) columns
N_CORES = 8

f32 = mybir.dt.float32
f16 = mybir.dt.float16
AF = mybir.ActivationFunctionType
ALU = mybir.AluOpType

_CH512 = [(c * 512, min(512, NP - c * 512)) for c in range((NP + 511) // 512)]
_CHNB = [(c * 512, min(512, NB - c * 512)) for c in range((NB + 511) // 512)]


def build_program(reps=1, loop_n=0, zb=True):
    """Build the SPMD Bass/Tile program. Returns compiled Bacc module."""
    nc = bacc.Bacc("TRN2", target_bir_lowering=False, debug=False)

    def inp(name, shape, dt):
        return nc.dram_tensor(name, list(shape), dt, kind="ExternalInput").ap()

    d_pclT = inp("pclT", (4, NB), f16)
    d_delta0 = inp("delta0", (4, NB), f16)
    d_Wf1 = inp("Wf1", (3, F), f16)
    d_bf1 = inp("bf1", (F, 1), f32)
    d_WfW = inp("WfW", (F, F), f16)
    d_bg = inp("bg", (F, 1), f32)
    d_W0g = inp("W0g", (3, F), f16)
    d_W0gn = inp("W0gn", (3, F), f16)
    d_I128 = inp("I128", (F, F), f16)
    d_Wb1 = inp("Wb1", (F, F), f16)
    d_Wb2 = inp("Wb2", (F, F), f16)
    d_pb1 = inp("pb1", (F, 1), f32)     # +b1
    d_nb1 = inp("nb1", (F, 1), f32)     # -b1
    d_pb2 = inp("pb2", (F, 1), f32)     # +b2eff
    d_nb2 = inp("nb2", (F, 1), f32)     # -b2eff
    d_WoS = inp("WoS", (F, 3 * STEPS), f16)
    d_bo4 = inp("bo4", (4, STEPS), f32)        # 4*bo_eff per step
    d_cboL = inp("cboL", (4, 2 * STEPS), f32)  # edge count corrections
    d_cboR = inp("cboR", (4, 2 * STEPS), f32)
    d_eL = inp("eL", (F, 3 * STEPS), f16)
    d_eLn = inp("eLn", (F, 3 * STEPS), f16)
    d_eR = inp("eR", (F, 3 * STEPS), f16)
    d_eRn = inp("eRn", (F, 3 * STEPS), f16)
    d_flagL = inp("flagL", (4, 1), f32)
    d_flagR = inp("flagR", (4, 1), f32)
    d_out = nc.dram_tensor("outT", [4, CHUNK], f16, kind="ExternalOutput").ap()

    from contextlib import ExitStack
    with tile.TileContext(nc) as tc, ExitStack() as ctx:
        cpool = ctx.enter_context(tc.tile_pool(name="const", bufs=1))
        hpool = ctx.enter_context(tc.tile_pool(name="h", bufs=4))
        tp2 = ctx.enter_context(tc.tile_pool(name="t2", bufs=5))
        tpool = ctx.enter_context(tc.tile_pool(name="tiny", bufs=2))
        psp = ctx.enter_context(tc.tile_pool(name="ps", bufs=3, space="PSUM"))
        psbc = ctx.enter_context(tc.tile_pool(name="psBC", bufs=2, space="PSUM"))
        pssc = ctx.enter_context(tc.tile_pool(name="psS", bufs=1, space="PSUM"))
        h0pool = ctx.enter_context(tc.tile_pool(name="h0p", bufs=14))
        h2pool = ctx.enter_context(tc.tile_pool(name="h2p", bufs=7))

        def load(dram, shape, dt, tag):
            t = cpool.tile(list(shape), dt, tag=tag)
            nc.sync.dma_start(t[:], dram[:])
            return t

        pclT = load(d_pclT, (4, NB), f16, "pclT")
        delta_a = load(d_delta0, (4, NB), f16, "delta_a")
        delta_b = load(d_delta0, (4, NB), f16, "delta_b")
        Wf1 = load(d_Wf1, (3, F), f16, "Wf1")
        bf1 = load(d_bf1, (F, 1), f32, "bf1")
        WfW = load(d_WfW, (F, F), f16, "WfW")
        bg = load(d_bg, (F, 1), f32, "bg")
        W0g = load(d_W0g, (3, F), f16, "W0g")
        W0gn = load(d_W0gn, (3, F), f16, "W0gn")
        I128 = load(d_I128, (F, F), f16, "I128")
        Wb1 = load(d_Wb1, (F, F), f16, "Wb1")
        Wb2 = load(d_Wb2, (F, F), f16, "Wb2")
        pb1 = load(d_pb1, (F, 1), f32, "pb1")
        nb1 = load(d_nb1, (F, 1), f32, "nb1")
        pb2 = load(d_pb2, (F, 1), f32, "pb2")
        nb2 = load(d_nb2, (F, 1), f32, "nb2")
        WoS = load(d_WoS, (F, 3 * STEPS), f16, "WoS")
        bo4 = load(d_bo4, (4, STEPS), f32, "bo4")
        cboL = load(d_cboL, (4, 2 * STEPS), f32, "cboL")
        cboR = load(d_cboR, (4, 2 * STEPS), f32, "cboR")
        eL = load(d_eL, (F, 3 * STEPS), f16, "eL")
        eLn = load(d_eLn, (F, 3 * STEPS), f16, "eLn")
        eR = load(d_eR, (F, 3 * STEPS), f16, "eR")
        eRn = load(d_eRn, (F, 3 * STEPS), f16, "eRn")
        flagL = load(d_flagL, (4, 1), f32, "flagL")
        flagR = load(d_flagR, (4, 1), f32, "flagR")

        Gk = cpool.tile([F, R4], f16, tag="Gk")
        h3_a = cpool.tile([F, R4], f16, tag="h3_a")
        h3_b = cpool.tile([F, R4], f16, tag="h3_b")
        A0e = cpool.tile([F, NB], f16, tag="A0e")
        G0 = cpool.tile([F, NP], f16, tag="G0")

        # scatter PSUM: one bank holding 3 rotating [3,512] slots at
        # partition offsets 0/32/64 (96 is not a legal base partition)
        ps_sc = pssc.tile([F, 512], f32, tag="S")
        sc_ctr = [0]

        def pairv(t, fd):
            # [P, 1024] tile -> strided pair view [P, 2, fd] (halves at
            # byte offsets 0 and 512 cols)
            return t[:].rearrange("p (h n) -> p h n", h=2)[:, :, 0:fd]

        # ---- greedy tri-engine balancer for elementwise work ----
        # constants calibrated against the TimelineSim instruction model
        load_ns = {"ACT": 0.0, "DVE": 0.0, "POOL": 0.0, "DMA": 0.0}

        def c_act(fd):
            return fd * 0.833 + 190

        def c_dve_ps(fd):
            return fd * 1.042 + 125

        def c_dve_stt_sb(fd):
            return fd * 1.042 + 60

        def c_dve_tt(fd):
            return fd * 0.521 + 60

        def c_pool_tt(fd):
            return fd * 1.984 + 120

        def pick(options):
            # options: list of (fn, [(eng, cost), ...])
            best = None
            for fn, usages in options:
                new = dict(load_ns)
                for e, c in usages:
                    new[e] += c
                key = (max(new.values()), sum(c for _, c in usages))
                if best is None or key < best[0]:
                    best = (key, fn, usages)
            for e, c in best[2]:
                load_ns[e] += c
            best[1]()

        def charge(eng, cost):
            load_ns[eng] += cost

        def relu_op(dst, src, fd, bias=None):
            # psum -> sbuf relu, optional per-partition bias. GPSIMD cannot
            # read PSUM, but a DMA drain (psum -> sbuf staging) lets Pool do
            # the relu as TT-max against zeros (bias-free case only).
            def on_act():
                nc.scalar.activation(dst, src, AF.Relu,
                                     bias=(bias[:, :] if bias is not None else 0.0))
            def on_dve():
                if bias is not None:
                    nc.vector.tensor_scalar(dst, src, bias[:, :], 0.0, ALU.add, ALU.max)
                else:
                    nc.vector.tensor_scalar_max(dst, src, 0.0)
            pick([(on_act, [("ACT", c_act(fd))]),
                  (on_dve, [("DVE", c_dve_ps(fd))])])

        def copy_op(dst, src, fd):
            def on_act():
                nc.scalar.activation(dst, src, AF.Copy)
            def on_dve():
                nc.vector.tensor_copy(dst, src)
            pick([(on_act, [("ACT", c_act(fd))]),
                  (on_dve, [("DVE", c_dve_ps(fd))])])

        def fused_resid_pair(dst_pair, ps_pair, hin_pair, fd, pb, nb):
            # dst = max(ps, -b) + hin over a k-pair (== hin + relu(ps+b) - b)
            # GPSIMD cannot read PSUM and only supports TensorTensor-class
            # ops: its 2-op path (ACT relu -> sbuf, Pool TT add) is exact
            # only when b == 0, so it is only emitted in zero-bias programs.
            fd2 = 2 * fd
            def on_dve():
                nc.vector.scalar_tensor_tensor(dst_pair, ps_pair, nb[:, :],
                                               hin_pair, ALU.max, ALU.add)
            def on_2op():
                t = tp2.tile([F, 1024], f16, tag="t2")
                tv = pairv(t, fd)
                nc.scalar.activation(tv, ps_pair, AF.Relu, bias=pb[:, :])
                if zb:
                    nc.vector.tensor_add(dst_pair, tv, hin_pair)
                else:
                    nc.vector.scalar_tensor_tensor(dst_pair, tv, nb[:, :],
                                                   hin_pair, ALU.add, ALU.add)
            def on_2op_pool():
                t = tp2.tile([F, 1024], f16, tag="t2")
                tv = pairv(t, fd)
                nc.scalar.activation(tv, ps_pair, AF.Relu)
                nc.gpsimd.tensor_add(dst_pair, tv, hin_pair)
            def on_dma():
                # relu straight into dst, then the residual add rides the
                # software-DGE accumulate path (idle DMA rings)
                nc.scalar.activation(dst_pair, ps_pair, AF.Relu)
                nc.gpsimd.dma_start(dst_pair, hin_pair, accum_op=ALU.add)
            opts = [(on_dve, [("DVE", c_dve_ps(fd2))]),
                    (on_2op, [("ACT", c_act(fd2)),
                              ("DVE", c_dve_tt(fd2) if zb else c_dve_stt_sb(fd2))])]
            if zb:
                opts.append(
                    (on_2op_pool, [("ACT", c_act(fd2)), ("POOL", c_pool_tt(fd2))]))
            pick(opts)

        def delta_update(dst, ps, din, fd, step):
            # dst = ps + 4*bo_eff[step] + din
            sc = bo4[0:3, step:step + 1]
            def on_dve():
                nc.vector.scalar_tensor_tensor(dst, ps, sc, din, ALU.add, ALU.add)
            def on_2op():
                t = hpool.tile([4, 512], f16, tag="t4")
                nc.scalar.activation(t[0:3, :fd], ps, AF.Identity, bias=sc)
                nc.vector.tensor_add(dst, t[0:3, :fd], din)
            def on_2op_pool():
                t = hpool.tile([4, 512], f16, tag="t4")
                nc.scalar.activation(t[0:3, :fd], ps, AF.Identity, bias=sc)
                nc.gpsimd.tensor_add(dst, t[0:3, :fd], din)
            def on_dma():
                nc.scalar.activation(dst, ps, AF.Identity, bias=sc)
                nc.gpsimd.dma_start(dst, din, accum_op=ALU.add)
            pick([(on_dve, [("DVE", c_dve_ps(fd))]),
                  (on_2op, [("ACT", c_act(fd)), ("DVE", c_dve_tt(fd))]),
                  (on_2op_pool, [("ACT", c_act(fd)), ("POOL", c_pool_tt(fd))])])

        # one column at the k=2/k=3 boundary is read (as cone garbage) by the
        # interleaved scatter before any tile writes it on step 0
        nc.vector.memset(h3_a[:, 3 * NP - 1:3 * NP], 0.0)
        nc.vector.memset(h3_b[:, 3 * NP - 1:3 * NP], 0.0)

        # ---------------- preamble: A0e, G0, Gk ----------------
        # per-tile interleave so step-0's early tiles unblock fast; Gk is
        # built with ONE 4-block strided TT per tile (the four k-shifts are
        # consecutive A0e columns, G0 broadcasts along the k dim)
        GkvF = Gk[:].rearrange("p (k n) -> p k n", k=K)

        def gk_quad(ci):
            c0, fd = _CH512[ci]
            dstq = GkvF[:, :, c0:c0 + fd]
            g0b = G0[:, c0:c0 + fd].unsqueeze(1).broadcast_to([F, K, fd])
            a0b = A0e[:, GW + OFF[0] + c0:GW + OFF[0] + c0 + fd]
            a0q = bass.AP(a0b.tensor, a0b.offset,
                          [list(a0b.ap[0]), [1, K], [1, fd]])
            def on_dve():
                nc.vector.tensor_add(dstq, g0b, a0q)
            def on_pool():
                nc.gpsimd.tensor_add(dstq, g0b, a0q)
            if ci < 2:
                on_dve()
                charge("DVE", c_dve_tt(K * fd))
            else:
                pick([(on_dve, [("DVE", c_dve_tt(K * fd))]),
                      (on_pool, [("POOL", c_pool_tt(K * fd))])])

        for ci in range(len(_CHNB) + 1):
            if ci < len(_CHNB):
                c0, fd = _CHNB[ci]
                ps = psp.tile([F, 512], f32, tag="ps")
                nc.tensor.matmul(ps[:, :fd], W0g[:, :], pclT[0:3, c0:c0 + fd],
                                 start=True, stop=True)
                copy_op(A0e[:, c0:c0 + fd], ps[:, :fd], fd)
            if ci < len(_CH512):
                c0, fd = _CH512[ci]
                ps = psp.tile([F, 512], f32, tag="ps")
                nc.tensor.matmul(ps[:, :fd], Wf1[:, :],
                                 pclT[0:3, GW + c0:GW + c0 + fd],
                                 start=True, stop=True)
                hf = hpool.tile([F, 512], f16, tag="h0")
                nc.scalar.activation(hf[:, :fd], ps[:, :fd], AF.Relu, bias=bf1[:, :])
                ps2 = psp.tile([F, 512], f32, tag="ps")
                nc.tensor.matmul(ps2[:, :fd], WfW[:, :], hf[:, :fd],
                                 start=True, stop=False)
                nc.tensor.matmul(ps2[:, :fd], W0gn[:, :],
                                 pclT[0:3, GW + c0:GW + c0 + fd],
                                 start=False, stop=True)
                nc.scalar.activation(G0[:, c0:c0 + fd], ps2[:, :fd], AF.Identity,
                                     bias=bg[:, :])
            if 0 <= ci - 1 < len(_CH512):
                gk_quad(ci - 1)

        # ---------------- langevin steps ----------------
        def emit_rep(final_rep):
            for step in range(STEPS):
                d_in = delta_a if step % 2 == 0 else delta_b
                d_out_t = delta_b if step % 2 == 0 else delta_a
                h3 = h3_a if step % 2 == 0 else h3_b
                h3kv = h3[:].rearrange("p (k n) -> p k n", k=K)
                final = (step == STEPS - 1) and final_rep
                s3 = slice(3 * step, 3 * step + 3)

                def emit_passA(cb):
                    c0, fd = _CH512[cb]
                    for kk in (0, 2):
                        h0p = h0pool.tile([F, 1024], f16, tag="h0")
                        for j in range(2):
                            k = kk + j
                            ps = psp.tile([F, 512], f32, tag="ps")
                            nc.tensor.matmul(
                                ps[:, :fd], W0g[:, :],
                                d_in[0:3, GW + OFF[k] + c0:GW + OFF[k] + c0 + fd],
                                start=True, stop=False)
                            nc.tensor.matmul(ps[:, :fd], I128[:, :],
                                             Gk[:, k * NP + c0:k * NP + c0 + fd],
                                             start=False, stop=True)
                            relu_op(h0p[:, 512 * j:512 * j + fd], ps[:, :fd], fd)
                        h0s[(kk, cb)] = h0p

                def emit_passB(cb):
                    c0, fd = _CH512[cb]
                    for kk in (0, 2):
                        h0p = h0s[(kk, cb)]
                        ps = psbc.tile([F, 1024], f32, tag="bc")
                        for j in range(2):
                            nc.tensor.matmul(ps[:, 512 * j:512 * j + fd], Wb1[:, :],
                                             h0p[:, 512 * j:512 * j + fd],
                                             start=True, stop=True)
                        h2p = h2pool.tile([F, 1024], f16, tag="h2")
                        fused_resid_pair(pairv(h2p, fd), pairv(ps, fd),
                                         pairv(h0p, fd), fd, pb1, nb1)
                        h2s[(kk, cb)] = h2p

                def emit_passC(cb):
                    c0, fd = _CH512[cb]
                    for kk in (0, 2):
                        h2p = h2s[(kk, cb)]
                        ps = psbc.tile([F, 1024], f32, tag="bc")
                        for j in range(2):
                            nc.tensor.matmul(ps[:, 512 * j:512 * j + fd], Wb2[:, :],
                                             h2p[:, 512 * j:512 * j + fd],
                                             start=True, stop=True)
                        fused_resid_pair(h3kv[:, kk:kk + 2, c0:c0 + fd],
                                         pairv(ps, fd), pairv(h2p, fd), fd, pb2, nb2)

                def mirror_fix(flag, src_l, dst_l0, w):
                    # mirror guards at global edges (flag=0 -> no-op on
                    # interior); handles w contiguous dst columns in one shot
                    # with a broadcast source
                    dst = d_out_t[0:3, GW + dst_l0:GW + dst_l0 + w]
                    src = d_out_t[0:3, GW + src_l:GW + src_l + 1].broadcast_to([3, w])
                    t = tpool.tile([4, 2], f16, tag="mir")
                    nc.vector.tensor_sub(t[0:3, :w], src, dst)
                    nc.vector.tensor_scalar_mul(t[0:3, :w], t[0:3, :w], flag[0:3, :])
                    nc.vector.tensor_add(dst, dst, t[0:3, :w])

                def emit_scatter(cb):
                    c0, fd = _CH512[cb]
                    p0 = 32 * (sc_ctr[0] % 3)
                    sc_ctr[0] += 1
                    ps = ps_sc[p0:p0 + 4, :]
                    mms = []
                    for k in range(K):
                        st = k * NP + c0 - OFF[k]
                        mms.append((ps[0:3, :fd], WoS[:, s3], h3[:, st:st + fd]))
                    if cb == 0:
                        pcol = ps[0:3, HALO:HALO + 1]
                        for col in (HALO, HALO + 1, NP + HALO):
                            mms.append((pcol, eL[:, s3], h3[:, col:col + 1]))
                        mms.append((pcol, eLn[:, s3],
                                    h3[:, 3 * NP + HALO - 1:3 * NP + HALO]))
                    if cb == len(_CH512) - 1:
                        lN = HALO + CHUNK - 1
                        pN = ps[0:3, lN - c0:lN - c0 + 1]
                        mms.append((pN, eR[:, s3], h3[:, 3 * NP + lN:3 * NP + lN + 1]))
                        for col in (lN + 2, NP + lN + 1):
                            mms.append((pN, eRn[:, s3], h3[:, col:col + 1]))
                        mms.append((ps[0:3, lN - 1 - c0:lN - c0], eRn[:, s3],
                                    h3[:, lN + 1:lN + 2]))
                    for i, (o, w, m) in enumerate(mms):
                        nc.tensor.matmul(o, w, m, start=(i == 0),
                                         stop=(i == len(mms) - 1))
                    delta_update(d_out_t[0:3, GW + c0:GW + c0 + fd], ps[0:3, :fd],
                                 d_in[0:3, GW + c0:GW + c0 + fd], fd, step)
                    if cb == 0:
                        nc.vector.tensor_add(
                            d_out_t[0:3, GW + HALO:GW + HALO + 2],
                            d_out_t[0:3, GW + HALO:GW + HALO + 2],
                            cboL[0:3, 2 * step:2 * step + 2])
                        charge("DVE", 80)
                    if cb == len(_CH512) - 1:
                        r0 = HALO + CHUNK - 2
                        nc.vector.tensor_add(
                            d_out_t[0:3, GW + r0:GW + r0 + 2],
                            d_out_t[0:3, GW + r0:GW + r0 + 2],
                            cboR[0:3, 2 * step:2 * step + 2])
                        charge("DVE", 80)

                h0s = {}
                h2s = {}
                nblk = len(_CH512)
                for cb in range(nblk + 5):
                    if cb < nblk:
                        emit_passA(cb)
                    if 0 <= cb - 2 < nblk:
                        emit_passB(cb - 2)
                    if 0 <= cb - 3 < nblk:
                        emit_passC(cb - 3)
                    if 0 <= cb - 5 < nblk:
                        emit_scatter(cb - 5)

                if final:
                    nc.sync.dma_start(
                        d_out[:, :], d_out_t[0:4, GW + HALO:GW + HALO + CHUNK])
                else:
                    mirror_fix(flagL, HALO, HALO - 2, 2)
                    mirror_fix(flagR, HALO + CHUNK - 1, HALO + CHUNK, 1)

        if loop_n:
            with tc.For_i(0, loop_n, 1):
                emit_rep(False)
            emit_rep(True)
        else:
            for rep in range(reps):
                emit_rep(rep == reps - 1)

    nc.compile()
    return nc


def host_prep(inputs):
    """Slice/transpose/pad inputs per core; build weight-variant constants."""
    pcl = np.asarray(inputs["pcl_noisy"], np.float32)
    Wf1 = np.asarray(inputs["Wf1"], np.float32)
    bf1 = np.asarray(inputs["bf1"], np.float32)
    Wf2 = np.asarray(inputs["Wf2"], np.float32)
    bf2 = np.asarray(inputs["bf2"], np.float32)
    W0 = np.asarray(inputs["W0"], np.float32)
    b0 = np.asarray(inputs["b0"], np.float32)
    Wb = np.asarray(inputs["Wb"], np.float32)
    bb = np.asarray(inputs["bb"], np.float32)
    Wo = np.asarray(inputs["Wo"], np.float32)
    bo = np.asarray(inputs["bo"], np.float32)

    W0g = W0[:3]
    WfW = Wf2 @ W0[3:]
    bg = bf2 @ W0[3:] + b0
    offs = np.arange(-(K - 1) // 2, (K - 1) // 2 + 1)
    nbr = np.clip(np.arange(N)[:, None] + offs, 0, N - 1).reshape(-1)
    counts = np.bincount(nbr, minlength=N).astype(np.float32)

    b1 = bb[0]
    b2eff = bb[1] + Wb[1].T @ b1
    bsum = b1 + b2eff
    svals = [S0 * DECAY ** i for i in range(STEPS)]
    WoS = np.concatenate([s * Wo for s in svals], axis=1)          # [128, 12]
    bo_eff = np.stack([s * (Wo.T @ bsum + bo) for s in svals], 1)  # [3, STEPS]
    bo4 = np.zeros((4, STEPS), np.float32)
    bo4[0:3] = 4.0 * bo_eff

    hf = np.float16
    shared = {
        "Wf1": Wf1.astype(hf), "bf1": bf1.reshape(F, 1),
        "WfW": WfW.astype(hf), "bg": bg.reshape(F, 1),
        "W0g": W0g.astype(hf), "W0gn": (-W0g).astype(hf),
        "I128": np.eye(F, dtype=np.float32).astype(hf),
        "Wb1": Wb[0].astype(hf), "Wb2": Wb[1].astype(hf),
        "pb1": b1.reshape(F, 1).astype(np.float32),
        "nb1": (-b1).reshape(F, 1).astype(np.float32),
        "pb2": b2eff.reshape(F, 1).astype(np.float32),
        "nb2": (-b2eff).reshape(F, 1).astype(np.float32),
        "WoS": WoS.astype(hf),
        "bo4": bo4,
    }
    zeros_e = np.zeros((F, 3 * STEPS), np.float16)
    zeros_c = np.zeros((4, 2 * STEPS), np.float32)
    # edge-count corrections: counts - 4 is nonzero only at global cols
    # {0, 1} (left) and {N-2, N-1} (right)
    cboL = np.zeros((4, 2 * STEPS), np.float32)
    cboR = np.zeros((4, 2 * STEPS), np.float32)
    for s in range(STEPS):
        cboL[0:3, 2 * s:2 * s + 2] = (counts[0:2] - 4.0) * bo_eff[:, s:s + 1]
        cboR[0:3, 2 * s:2 * s + 2] = (counts[N - 2:N] - 4.0) * bo_eff[:, s:s + 1]

    in_maps = []
    for core in range(N_CORES):
        b, ch = core // 4, core % 4
        g0 = ch * CHUNK - HALO
        idx = np.clip(np.arange(g0 - GW, g0 + NP + GW), 0, N - 1)
        pclT = np.empty((4, NB), np.float16)
        pclT[0:3] = pcl[b, idx].T.astype(np.float16)
        pclT[3] = 0.0
        delta0 = np.zeros((4, NB), np.float16)
        isL, isR = ch == 0, ch == 3
        m = dict(shared)
        m["pclT"] = pclT
        m["delta0"] = delta0
        m["eL"] = (WoS.astype(hf) if isL else zeros_e)
        m["eLn"] = ((-WoS).astype(hf) if isL else zeros_e)
        m["eR"] = (WoS.astype(hf) if isR else zeros_e)
        m["eRn"] = ((-WoS).astype(hf) if isR else zeros_e)
        m["cboL"] = (cboL if isL else zeros_c)
        m["cboR"] = (cboR if isR else zeros_c)
        m["flagL"] = np.full((4, 1), 1.0 if isL else 0.0, np.float32)
        m["flagR"] = np.full((4, 1), 1.0 if isR else 0.0, np.float32)
        in_maps.append(m)
    return in_maps


_CACHED = {}


def _get_program(reps=1, zb=True):
    key = (reps, zb)
    if key not in _CACHED:
        _CACHED[key] = build_program(reps, zb=zb)
    return _CACHED[key]


def kernel(**inputs):
    zb = all(
        not np.any(np.asarray(inputs[k]))
        for k in ("bf1", "bf2", "b0", "bb", "bo"))
    nc = _get_program(1, zb)
    in_maps = host_prep(inputs)
    res = run_bass_kernel_spmd(nc, in_maps, list(range(N_CORES)))
    pcl = np.asarray(inputs["pcl_noisy"], np.float32)
    out = np.empty((B, N, D), np.float32)
    for core in range(N_CORES):
        b, ch = core // 4, core % 4
        sl = slice(ch * CHUNK, (ch + 1) * CHUNK)
        out[b, sl] = pcl[b, sl] + res.results[core]["outT"][0:3].T.astype(np.float32)
    return out


# revision 49
# speedup vs baseline: 1.3173x; 1.2741x over previous
"""Trainium2 Bass kernel for nn_DenoiseNet (langevin point-cloud denoiser).

Strategy (8 NeuronCores, SPMD, zero inter-core communication):
  - Shard over B(2) x 4 contiguous N-chunks of 4096 points, each core padded
    with a 64-point halo on both sides (dependency cone grows 3 pts/step,
    4 steps -> 12 needed). Global-edge clipping handled exactly via per-core
    weight data (zeros on interior cores), so one program runs on all cores.
  - Feature-major fp16 layout [128 feat, (k, n) cols]. Sliding-window gather
    and scatter_add become free-dim shifted access patterns; the scatter
    k-sum rides matmul PSUM accumulation.
  - Residual adds are fused into scalar_tensor_tensor ops with a bias
    refold: h2' = max(ps1, -b1) + h0 equals (h0 + relu(ps1+b1)) - b1, and
    the -b1 shift is folded into the next layer's bias and the output
    constant. The per-column +bo constant is applied in the delta-update
    STT (with tiny edge-count corrections at the two global boundaries).
  - block1/block2 use k-paired [128,1024] PSUM tiles spanning two banks so
    one elementwise op drains two matmul outputs (strided pair APs via
    AP.rearrange); the scatter PSUM packs 4 rotating [3,512] slots into one
    bank at partition offsets 0/32/64.
  - Elementwise work is greedily load-balanced across ACT/DVE/GPSIMD using
    cost constants calibrated against the TimelineSim instruction model.
"""

import sys
import numpy as np

for _p in ("/opt/trn_rl_repo",):
    if _p not in sys.path:
        sys.path.insert(0, _p)

import concourse.bass as bass
import concourse.bacc as bacc
import concourse.tile as tile
from concourse import mybir
from concourse.bass_utils import run_bass_kernel_spmd

# ---- problem constants (hardcoded per harness contract) ----
B, N, D = 2, 16384, 3
F = 128
K = 4
OFF = [-2, -1, 0, 1]
STEPS, S0, DECAY = 4, 0.2, 0.95
CHUNK, HALO, GW = 4096, 64, 2
NP = CHUNK + 2 * HALO          # 4224 local points
NB = NP + 2 * GW               # 4228 buffer cols (with guards)
R4 = K * NP                    # 16896 (k,n) columns
N_CORES = 8

f32 = mybir.dt.float32
f16 = mybir.dt.float16
AF = mybir.ActivationFunctionType
ALU = mybir.AluOpType

_CH512 = [(c * 512, min(512, NP - c * 512)) for c in range((NP + 511) // 512)]
_CHNB = [(c * 512, min(512, NB - c * 512)) for c in range((NB + 511) // 512)]


def build_program(reps=1, loop_n=0, zb=True):
    """Build the SPMD Bass/Tile program. Returns compiled Bacc module."""
    nc = bacc.Bacc("TRN2", target_bir_lowering=False, debug=False)

    def inp(name, shape, dt):
        return nc.dram_tensor(name, list(shape), dt, kind="ExternalInput").ap()

    d_pclT = inp("pclT", (4, NB), f16)
    d_delta0 = inp("delta0", (4, NB), f16)
    d_Wf1 = inp("Wf1", (3, F), f16)
    d_bf1 = inp("bf1", (F, 1), f32)
    d_WfW = inp("WfW", (F, F), f16)
    d_bg = inp("bg", (F, 1), f32)
    d_W0g = inp("W0g", (3, F), f16)
    d_W0gn = inp("W0gn", (3, F), f16)
    d_I128 = inp("I128", (F, F), f16)
    d_Wb1 = inp("Wb1", (F, F), f16)
    d_Wb2 = inp("Wb2", (F, F), f16)
    d_pb1 = inp("pb1", (F, 1), f32)     # +b1
    d_nb1 = inp("nb1", (F, 1), f32)     # -b1
    d_pb2 = inp("pb2", (F, 1), f32)     # +b2eff
    d_nb2 = inp("nb2", (F, 1), f32)     # -b2eff
    d_WoS = inp("WoS", (F, 3 * STEPS), f16)
    d_bo4 = inp("bo4", (4, STEPS), f32)        # 4*bo_eff per step
    d_cboL = inp("cboL", (4, 2 * STEPS), f32)  # edge count corrections
    d_cboR = inp("cboR", (4, 2 * STEPS), f32)
    d_eL = inp("eL", (F, 3 * STEPS), f16)
    d_eLn = inp("eLn", (F, 3 * STEPS), f16)
    d_eR = inp("eR", (F, 3 * STEPS), f16)
    d_eRn = inp("eRn", (F, 3 * STEPS), f16)
    d_flagL = inp("flagL", (4, 1), f32)
    d_flagR = inp("flagR", (4, 1), f32)
    d_out = nc.dram_tensor("outT", [4, CHUNK], f16, kind="ExternalOutput").ap()

    from contextlib import ExitStack
    with tile.TileContext(nc) as tc, ExitStack() as ctx:
        cpool = ctx.enter_context(tc.tile_pool(name="const", bufs=1))
        hpool = ctx.enter_context(tc.tile_pool(name="h", bufs=4))
        tp2 = ctx.enter_context(tc.tile_pool(name="t2", bufs=6))
        tpool = ctx.enter_context(tc.tile_pool(name="tiny", bufs=2))
        psp = ctx.enter_context(tc.tile_pool(name="ps", bufs=3, space="PSUM"))
        psbc = ctx.enter_context(tc.tile_pool(name="psBC", bufs=2, space="PSUM"))
        pssc = ctx.enter_context(tc.tile_pool(name="psS", bufs=1, space="PSUM"))
        h0pool = ctx.enter_context(tc.tile_pool(name="h0p", bufs=12))
        h2pool = ctx.enter_context(tc.tile_pool(name="h2p", bufs=8))

        def load(dram, shape, dt, tag):
            t = cpool.tile(list(shape), dt, tag=tag)
            nc.sync.dma_start(t[:], dram[:])
            return t

        pclT = load(d_pclT, (4, NB), f16, "pclT")
        delta_a = load(d_delta0, (4, NB), f16, "delta_a")
        delta_b = load(d_delta0, (4, NB), f16, "delta_b")
        Wf1 = load(d_Wf1, (3, F), f16, "Wf1")
        bf1 = load(d_bf1, (F, 1), f32, "bf1")
        WfW = load(d_WfW, (F, F), f16, "WfW")
        bg = load(d_bg, (F, 1), f32, "bg")
        W0g = load(d_W0g, (3, F), f16, "W0g")
        W0gn = load(d_W0gn, (3, F), f16, "W0gn")
        I128 = load(d_I128, (F, F), f16, "I128")
        Wb1 = load(d_Wb1, (F, F), f16, "Wb1")
        Wb2 = load(d_Wb2, (F, F), f16, "Wb2")
        pb1 = load(d_pb1, (F, 1), f32, "pb1")
        nb1 = load(d_nb1, (F, 1), f32, "nb1")
        pb2 = load(d_pb2, (F, 1), f32, "pb2")
        nb2 = load(d_nb2, (F, 1), f32, "nb2")
        WoS = load(d_WoS, (F, 3 * STEPS), f16, "WoS")
        bo4 = load(d_bo4, (4, STEPS), f32, "bo4")
        cboL = load(d_cboL, (4, 2 * STEPS), f32, "cboL")
        cboR = load(d_cboR, (4, 2 * STEPS), f32, "cboR")
        eL = load(d_eL, (F, 3 * STEPS), f16, "eL")
        eLn = load(d_eLn, (F, 3 * STEPS), f16, "eLn")
        eR = load(d_eR, (F, 3 * STEPS), f16, "eR")
        eRn = load(d_eRn, (F, 3 * STEPS), f16, "eRn")
        flagL = load(d_flagL, (4, 1), f32, "flagL")
        flagR = load(d_flagR, (4, 1), f32, "flagR")

        Gk = cpool.tile([F, R4], f16, tag="Gk")
        h3_a = cpool.tile([F, R4], f16, tag="h3_a")
        h3_b = cpool.tile([F, R4], f16, tag="h3_b")
        A0e = cpool.tile([F, NB], f16, tag="A0e")
        G0 = cpool.tile([F, NP], f16, tag="G0")

        # scatter PSUM: one bank holding 3 rotating [3,512] slots at
        # partition offsets 0/32/64 (96 is not a legal base partition)
        ps_sc = pssc.tile([F, 512], f32, tag="S")
        sc_ctr = [0]
        rs_ctr = [0]

        def pairv(t, fd):
            # [P, 1024] tile -> strided pair view [P, 2, fd] (halves at
            # byte offsets 0 and 512 cols)
            return t[:].rearrange("p (h n) -> p h n", h=2)[:, :, 0:fd]

        # ---- greedy tri-engine balancer for elementwise work ----
        # constants calibrated against the TimelineSim instruction model
        load_ns = {"ACT": 0.0, "DVE": 0.0, "POOL": 0.0, "DMA": 0.0}

        def c_act(fd):
            return fd * 0.833 + 190

        def c_dve_ps(fd):
            return fd * 1.042 + 125

        def c_dve_stt_sb(fd):
            return fd * 1.042 + 60

        def c_dve_tt(fd):
            return fd * 0.521 + 60

        def c_pool_tt(fd):
            return fd * 1.984 + 120

        def pick(options):
            # options: list of (fn, [(eng, cost), ...])
            best = None
            for fn, usages in options:
                new = dict(load_ns)
                for e, c in usages:
                    new[e] += c
                key = (max(new.values()), sum(c for _, c in usages))
                if best is None or key < best[0]:
                    best = (key, fn, usages)
            for e, c in best[2]:
                load_ns[e] += c
            best[1]()

        def charge(eng, cost):
            load_ns[eng] += cost

        def relu_op(dst, src, fd, bias=None):
            # psum -> sbuf relu, optional per-partition bias. GPSIMD cannot
            # read PSUM, but a DMA drain (psum -> sbuf staging) lets Pool do
            # the relu as TT-max against zeros (bias-free case only).
            def on_act():
                nc.scalar.activation(dst, src, AF.Relu,
                                     bias=(bias[:, :] if bias is not None else 0.0))
            def on_dve():
                if bias is not None:
                    nc.vector.tensor_scalar(dst, src, bias[:, :], 0.0, ALU.add, ALU.max)
                else:
                    nc.vector.tensor_scalar_max(dst, src, 0.0)
            pick([(on_act, [("ACT", c_act(fd))]),
                  (on_dve, [("DVE", c_dve_ps(fd))])])

        def copy_op(dst, src, fd):
            def on_act():
                nc.scalar.activation(dst, src, AF.Copy)
            def on_dve():
                nc.vector.tensor_copy(dst, src)
            pick([(on_act, [("ACT", c_act(fd))]),
                  (on_dve, [("DVE", c_dve_ps(fd))])])

        def fused_resid_pair(dst_pair, ps_pair, hin_pair, fd, pb, nb):
            # dst = max(ps, -b) + hin over a k-pair (== hin + relu(ps+b) - b)
            # GPSIMD cannot read PSUM and only supports TensorTensor-class
            # ops: its 2-op path (ACT relu -> sbuf, Pool TT add) is exact
            # only when b == 0, so it is only emitted in zero-bias programs.
            fd2 = 2 * fd
            def on_dve():
                nc.vector.scalar_tensor_tensor(dst_pair, ps_pair, nb[:, :],
                                               hin_pair, ALU.max, ALU.add)
            def on_2op():
                t = tp2.tile([F, 1024], f16, tag="t2")
                tv = pairv(t, fd)
                nc.scalar.activation(tv, ps_pair, AF.Relu, bias=pb[:, :])
                if zb:
                    nc.vector.tensor_add(dst_pair, tv, hin_pair)
                else:
                    nc.vector.scalar_tensor_tensor(dst_pair, tv, nb[:, :],
                                                   hin_pair, ALU.add, ALU.add)
            def on_2op_pool():
                t = tp2.tile([F, 1024], f16, tag="t2")
                tv = pairv(t, fd)
                nc.scalar.activation(tv, ps_pair, AF.Relu)
                nc.gpsimd.tensor_add(dst_pair, tv, hin_pair)
            opts = [(on_dve, [("DVE", c_dve_ps(fd2))]),
                    (on_2op, [("ACT", c_act(fd2)),
                              ("DVE", c_dve_tt(fd2) if zb else c_dve_stt_sb(fd2))])]
            if zb:
                opts.append(
                    (on_2op_pool, [("ACT", c_act(fd2)), ("POOL", c_pool_tt(fd2))]))
            pick(opts)

        def delta_update(dst, ps, din, fd, step):
            # dst = ps + 4*bo_eff[step] + din
            sc = bo4[0:3, step:step + 1]
            def on_dve():
                nc.vector.scalar_tensor_tensor(dst, ps, sc, din, ALU.add, ALU.add)
            def on_2op():
                t = hpool.tile([4, 512], f16, tag="t4")
                nc.scalar.activation(t[0:3, :fd], ps, AF.Identity, bias=sc)
                nc.vector.tensor_add(dst, t[0:3, :fd], din)
            def on_2op_pool():
                t = hpool.tile([4, 512], f16, tag="t4")
                nc.scalar.activation(t[0:3, :fd], ps, AF.Identity, bias=sc)
                nc.gpsimd.tensor_add(dst, t[0:3, :fd], din)
            pick([(on_dve, [("DVE", c_dve_ps(fd))]),
                  (on_2op, [("ACT", c_act(fd)), ("DVE", c_dve_tt(fd))]),
                  (on_2op_pool, [("ACT", c_act(fd)), ("POOL", c_pool_tt(fd))])])

        # one column at the k=2/k=3 boundary is read (as cone garbage) by the
        # interleaved scatter before any tile writes it on step 0
        nc.vector.memset(h3_a[:, 3 * NP - 1:3 * NP], 0.0)
        nc.vector.memset(h3_b[:, 3 * NP - 1:3 * NP], 0.0)

        # ---------------- preamble: A0e, G0, Gk ----------------
        # per-tile interleave so step-0's early tiles unblock fast; Gk is
        # built with ONE 4-block strided TT per tile (the four k-shifts are
        # consecutive A0e columns, G0 broadcasts along the k dim)
        GkvF = Gk[:].rearrange("p (k n) -> p k n", k=K)

        def gk_quad(ci):
            c0, fd = _CH512[ci]
            dstq = GkvF[:, :, c0:c0 + fd]
            g0b = G0[:, c0:c0 + fd].unsqueeze(1).broadcast_to([F, K, fd])
            a0b = A0e[:, GW + OFF[0] + c0:GW + OFF[0] + c0 + fd]
            a0q = bass.AP(a0b.tensor, a0b.offset,
                          [list(a0b.ap[0]), [1, K], [1, fd]])
            def on_dve():
                nc.vector.tensor_add(dstq, g0b, a0q)
            def on_pool():
                nc.gpsimd.tensor_add(dstq, g0b, a0q)
            if ci < 2:
                on_dve()
                charge("DVE", c_dve_tt(K * fd))
            else:
                pick([(on_dve, [("DVE", c_dve_tt(K * fd))]),
                      (on_pool, [("POOL", c_pool_tt(K * fd))])])

        for ci in range(len(_CHNB) + 1):
            if ci < len(_CHNB):
                c0, fd = _CHNB[ci]
                ps = psp.tile([F, 512], f32, tag="ps")
                nc.tensor.matmul(ps[:, :fd], W0g[:, :], pclT[0:3, c0:c0 + fd],
                                 start=True, stop=True)
                copy_op(A0e[:, c0:c0 + fd], ps[:, :fd], fd)
            if ci < len(_CH512):
                c0, fd = _CH512[ci]
                ps = psp.tile([F, 512], f32, tag="ps")
                nc.tensor.matmul(ps[:, :fd], Wf1[:, :],
                                 pclT[0:3, GW + c0:GW + c0 + fd],
                                 start=True, stop=True)
                hf = hpool.tile([F, 512], f16, tag="h0")
                nc.scalar.activation(hf[:, :fd], ps[:, :fd], AF.Relu, bias=bf1[:, :])
                ps2 = psp.tile([F, 512], f32, tag="ps")
                nc.tensor.matmul(ps2[:, :fd], WfW[:, :], hf[:, :fd],
                                 start=True, stop=False)
                nc.tensor.matmul(ps2[:, :fd], W0gn[:, :],
                                 pclT[0:3, GW + c0:GW + c0 + fd],
                                 start=False, stop=True)
                nc.scalar.activation(G0[:, c0:c0 + fd], ps2[:, :fd], AF.Identity,
                                     bias=bg[:, :])
            if 0 <= ci - 1 < len(_CH512):
                gk_quad(ci - 1)

        # ---------------- langevin steps ----------------
        def emit_rep(final_rep):
            for step in range(STEPS):
                d_in = delta_a if step % 2 == 0 else delta_b
                d_out_t = delta_b if step % 2 == 0 else delta_a
                h3 = h3_a if step % 2 == 0 else h3_b
                h3kv = h3[:].rearrange("p (k n) -> p k n", k=K)
                final = (step == STEPS - 1) and final_rep
                s3 = slice(3 * step, 3 * step + 3)

                def emit_passA(cb, kks=(0, 2)):
                    c0, fd = _CH512[cb]
                    for kk in kks:
                        h0p = h0pool.tile([F, 1024], f16, tag="h0")
                        for j in range(2):
                            k = kk + j
                            ps = psp.tile([F, 512], f32, tag="ps")
                            nc.tensor.matmul(ps[:, :fd], I128[:, :],
                                             Gk[:, k * NP + c0:k * NP + c0 + fd],
                                             start=True, stop=False)
                            nc.tensor.matmul(
                                ps[:, :fd], W0g[:, :],
                                d_in[0:3, GW + OFF[k] + c0:GW + OFF[k] + c0 + fd],
                                start=False, stop=True)
                            relu_op(h0p[:, 512 * j:512 * j + fd], ps[:, :fd], fd)
                        h0s[(kk, cb)] = h0p

                def emit_passB(cb, kks=(0, 2)):
                    c0, fd = _CH512[cb]
                    for kk in kks:
                        h0p = h0s[(kk, cb)]
                        ps = psbc.tile([F, 1024], f32, tag="bc")
                        for j in range(2):
                            nc.tensor.matmul(ps[:, 512 * j:512 * j + fd], Wb1[:, :],
                                             h0p[:, 512 * j:512 * j + fd],
                                             start=True, stop=True)
                        h2p = h2pool.tile([F, 1024], f16, tag="h2")
                        fused_resid_pair(pairv(h2p, fd), pairv(ps, fd),
                                         pairv(h0p, fd), fd, pb1, nb1)
                        h2s[(kk, cb)] = h2p

                def emit_passC(cb, kks=(0, 2)):
                    c0, fd = _CH512[cb]
                    for kk in kks:
                        h2p = h2s[(kk, cb)]
                        ps = psbc.tile([F, 1024], f32, tag="bc")
                        for j in range(2):
                            nc.tensor.matmul(ps[:, 512 * j:512 * j + fd], Wb2[:, :],
                                             h2p[:, 512 * j:512 * j + fd],
                                             start=True, stop=True)
                        fused_resid_pair(h3kv[:, kk:kk + 2, c0:c0 + fd],
                                         pairv(ps, fd), pairv(h2p, fd), fd, pb2, nb2)

                def mirror_fix(flag, src_l, dst_l0, w):
                    # mirror guards at global edges (flag=0 -> no-op on
                    # interior); handles w contiguous dst columns in one shot
                    # with a broadcast source
                    dst = d_out_t[0:3, GW + dst_l0:GW + dst_l0 + w]
                    src = d_out_t[0:3, GW + src_l:GW + src_l + 1].broadcast_to([3, w])
                    t = tpool.tile([4, 2], f16, tag="mir")
                    nc.vector.tensor_sub(t[0:3, :w], src, dst)
                    nc.vector.tensor_scalar_mul(t[0:3, :w], t[0:3, :w], flag[0:3, :])
                    nc.vector.tensor_add(dst, dst, t[0:3, :w])

                def emit_scatter(cb):
                    c0, fd = _CH512[cb]
                    p0 = 32 * (sc_ctr[0] % 3)
                    sc_ctr[0] += 1
                    ps = ps_sc[p0:p0 + 4, :]
                    mms = []
                    for k in range(K):
                        st = k * NP + c0 - OFF[k]
                        mms.append((ps[0:3, :fd], WoS[:, s3], h3[:, st:st + fd]))
                    if cb == 0:
                        pcol = ps[0:3, HALO:HALO + 1]
                        for col in (HALO, HALO + 1, NP + HALO):
                            mms.append((pcol, eL[:, s3], h3[:, col:col + 1]))
                        mms.append((pcol, eLn[:, s3],
                                    h3[:, 3 * NP + HALO - 1:3 * NP + HALO]))
                    if cb == len(_CH512) - 1:
                        lN = HALO + CHUNK - 1
                        pN = ps[0:3, lN - c0:lN - c0 + 1]
                        mms.append((pN, eR[:, s3], h3[:, 3 * NP + lN:3 * NP + lN + 1]))
                        for col in (lN + 2, NP + lN + 1):
                            mms.append((pN, eRn[:, s3], h3[:, col:col + 1]))
                        mms.append((ps[0:3, lN - 1 - c0:lN - c0], eRn[:, s3],
                                    h3[:, lN + 1:lN + 2]))
                    for i, (o, w, m) in enumerate(mms):
                        nc.tensor.matmul(o, w, m, start=(i == 0),
                                         stop=(i == len(mms) - 1))
                    delta_update(d_out_t[0:3, GW + c0:GW + c0 + fd], ps[0:3, :fd],
                                 d_in[0:3, GW + c0:GW + c0 + fd], fd, step)
                    if cb == 0:
                        nc.vector.tensor_add(
                            d_out_t[0:3, GW + HALO:GW + HALO + 2],
                            d_out_t[0:3, GW + HALO:GW + HALO + 2],
                            cboL[0:3, 2 * step:2 * step + 2])
                        charge("DVE", 80)
                    if cb == len(_CH512) - 1:
                        r0 = HALO + CHUNK - 2
                        nc.vector.tensor_add(
                            d_out_t[0:3, GW + r0:GW + r0 + 2],
                            d_out_t[0:3, GW + r0:GW + r0 + 2],
                            cboR[0:3, 2 * step:2 * step + 2])
                        charge("DVE", 80)

                h0s = {}
                h2s = {}
                nblk = len(_CH512)
                for cb in range(nblk + 4):
                    for kks in ((0,), (2,)):
                        if cb < nblk:
                            emit_passA(cb, kks)
                        if 0 <= cb - 3 < nblk:
                            emit_passC(cb - 3, kks)
                        if 0 <= cb - 1 < nblk:
                            emit_passB(cb - 1, kks)
                    if 0 <= cb - 4 < nblk:
                        emit_scatter(cb - 4)

                if final:
                    nc.sync.dma_start(
                        d_out[:, :], d_out_t[0:4, GW + HALO:GW + HALO + CHUNK])
                else:
                    mirror_fix(flagL, HALO, HALO - 2, 2)
                    mirror_fix(flagR, HALO + CHUNK - 1, HALO + CHUNK, 1)

        if loop_n:
            with tc.For_i(0, loop_n, 1):
                emit_rep(False)
            emit_rep(True)
        else:
            for rep in range(reps):
                emit_rep(rep == reps - 1)

    nc.compile()
    return nc


def host_prep(inputs):
    """Slice/transpose/pad inputs per core; build weight-variant constants."""
    pcl = np.asarray(inputs["pcl_noisy"], np.float32)
    Wf1 = np.asarray(inputs["Wf1"], np.float32)
    bf1 = np.asarray(inputs["bf1"], np.float32)
    Wf2 = np.asarray(inputs["Wf2"], np.float32)
    bf2 = np.asarray(inputs["bf2"], np.float32)
    W0 = np.asarray(inputs["W0"], np.float32)
    b0 = np.asarray(inputs["b0"], np.float32)
    Wb = np.asarray(inputs["Wb"], np.float32)
    bb = np.asarray(inputs["bb"], np.float32)
    Wo = np.asarray(inputs["Wo"], np.float32)
    bo = np.asarray(inputs["bo"], np.float32)

    W0g = W0[:3]
    WfW = Wf2 @ W0[3:]
    bg = bf2 @ W0[3:] + b0
    offs = np.arange(-(K - 1) // 2, (K - 1) // 2 + 1)
    nbr = np.clip(np.arange(N)[:, None] + offs, 0, N - 1).reshape(-1)
    counts = np.bincount(nbr, minlength=N).astype(np.float32)

    b1 = bb[0]
    b2eff = bb[1] + Wb[1].T @ b1
    bsum = b1 + b2eff
    svals = [S0 * DECAY ** i for i in range(STEPS)]
    WoS = np.concatenate([s * Wo for s in svals], axis=1)          # [128, 12]
    bo_eff = np.stack([s * (Wo.T @ bsum + bo) for s in svals], 1)  # [3, STEPS]
    bo4 = np.zeros((4, STEPS), np.float32)
    bo4[0:3] = 4.0 * bo_eff

    hf = np.float16
    shared = {
        "Wf1": Wf1.astype(hf), "bf1": bf1.reshape(F, 1),
        "WfW": WfW.astype(hf), "bg": bg.reshape(F, 1),
        "W0g": W0g.astype(hf), "W0gn": (-W0g).astype(hf),
        "I128": np.eye(F, dtype=np.float32).astype(hf),
        "Wb1": Wb[0].astype(hf), "Wb2": Wb[1].astype(hf),
        "pb1": b1.reshape(F, 1).astype(np.float32),
        "nb1": (-b1).reshape(F, 1).astype(np.float32),
        "pb2": b2eff.reshape(F, 1).astype(np.float32),
        "nb2": (-b2eff).reshape(F, 1).astype(np.float32),
        "WoS": WoS.astype(hf),
        "bo4": bo4,
    }
    zeros_e = np.zeros((F, 3 * STEPS), np.float16)
    zeros_c = np.zeros((4, 2 * STEPS), np.float32)
    # edge-count corrections: counts - 4 is nonzero only at global cols
    # {0, 1} (left) and {N-2, N-1} (right)
    cboL = np.zeros((4, 2 * STEPS), np.float32)
    cboR = np.zeros((4, 2 * STEPS), np.float32)
    for s in range(STEPS):
        cboL[0:3, 2 * s:2 * s + 2] = (counts[0:2] - 4.0) * bo_eff[:, s:s + 1]
        cboR[0:3, 2 * s:2 * s + 2] = (counts[N - 2:N] - 4.0) * bo_eff[:, s:s + 1]

    in_maps = []
    for core in range(N_CORES):
        b, ch = core // 4, core % 4
        g0 = ch * CHUNK - HALO
        idx = np.clip(np.arange(g0 - GW, g0 + NP + GW), 0, N - 1)
        pclT = np.empty((4, NB), np.float16)
        pclT[0:3] = pcl[b, idx].T.astype(np.float16)
        pclT[3] = 0.0
        delta0 = np.zeros((4, NB), np.float16)
        isL, isR = ch == 0, ch == 3
        m = dict(shared)
        m["pclT"] = pclT
        m["delta0"] = delta0
        m["eL"] = (WoS.astype(hf) if isL else zeros_e)
        m["eLn"] = ((-WoS).astype(hf) if isL else zeros_e)
        m["eR"] = (WoS.astype(hf) if isR else zeros_e)
        m["eRn"] = ((-WoS).astype(hf) if isR else zeros_e)
        m["cboL"] = (cboL if isL else zeros_c)
        m["cboR"] = (cboR if isR else zeros_c)
        m["flagL"] = np.full((4, 1), 1.0 if isL else 0.0, np.float32)
        m["flagR"] = np.full((4, 1), 1.0 if isR else 0.0, np.float32)
        in_maps.append(m)
    return in_maps


_CACHED = {}


def _get_program(reps=1, zb=True):
    key = (reps, zb)
    if key not in _CACHED:
        _CACHED[key] = build_program(reps, zb=zb)
    return _CACHED[key]


def kernel(**inputs):
    zb = all(
        not np.any(np.asarray(inputs[k]))
        for k in ("bf1", "bf2", "b0", "bb", "bo"))
    nc = _get_program(1, zb)
    in_maps = host_prep(inputs)
    res = run_bass_kernel_spmd(nc, in_maps, list(range(N_CORES)))
    pcl = np.asarray(inputs["pcl_noisy"], np.float32)
    out = np.empty((B, N, D), np.float32)
    for core in range(N_CORES):
        b, ch = core // 4, core % 4
        sl = slice(ch * CHUNK, (ch + 1) * CHUNK)
        out[b, sl] = pcl[b, sl] + res.results[core]["outT"][0:3].T.astype(np.float32)
    return out


# revision 54
# speedup vs baseline: 1.3479x; 1.0233x over previous
"""Trainium2 Bass kernel for nn_DenoiseNet (langevin point-cloud denoiser).

Strategy (8 NeuronCores, SPMD, zero inter-core communication):
  - Shard over B(2) x 4 contiguous N-chunks of 4096 points, each core padded
    with a 64-point halo on both sides (dependency cone grows 3 pts/step,
    4 steps -> 12 needed). Global-edge clipping handled exactly via per-core
    weight data (zeros on interior cores), so one program runs on all cores.
  - Feature-major fp16 layout [128 feat, (k, n) cols]. Sliding-window gather
    and scatter_add become free-dim shifted access patterns; the scatter
    k-sum rides matmul PSUM accumulation.
  - Residual adds are fused into scalar_tensor_tensor ops with a bias
    refold: h2' = max(ps1, -b1) + h0 equals (h0 + relu(ps1+b1)) - b1, and
    the -b1 shift is folded into the next layer's bias and the output
    constant. The per-column +bo constant is applied in the delta-update
    STT (with tiny edge-count corrections at the two global boundaries).
  - block1/block2 use k-paired [128,1024] PSUM tiles spanning two banks so
    one elementwise op drains two matmul outputs (strided pair APs via
    AP.rearrange); the scatter PSUM packs 4 rotating [3,512] slots into one
    bank at partition offsets 0/32/64.
  - Elementwise work is greedily load-balanced across ACT/DVE/GPSIMD using
    cost constants calibrated against the TimelineSim instruction model.
"""

import sys
import numpy as np

for _p in ("/opt/trn_rl_repo",):
    if _p not in sys.path:
        sys.path.insert(0, _p)

import concourse.bass as bass
import concourse.bacc as bacc
import concourse.tile as tile
from concourse import mybir
from concourse.bass_utils import run_bass_kernel_spmd

# ---- problem constants (hardcoded per harness contract) ----
B, N, D = 2, 16384, 3
F = 128
K = 4
OFF = [-2, -1, 0, 1]
STEPS, S0, DECAY = 4, 0.2, 0.95
CHUNK, HALO, GW = 4096, 64, 2
NP = CHUNK + 2 * HALO          # 4224 local points
NB = NP + 2 * GW               # 4228 buffer cols (with guards)
R4 = K * NP                    # 16896 (k,n) columns
N_CORES = 8

f32 = mybir.dt.float32
f16 = mybir.dt.float16
AF = mybir.ActivationFunctionType
ALU = mybir.AluOpType

_CH512 = [(c * 512, min(512, NP - c * 512)) for c in range((NP + 511) // 512)]
_CHNB = [(c * 512, min(512, NB - c * 512)) for c in range((NB + 511) // 512)]


def build_program(reps=1, loop_n=0, zb=True):
    """Build the SPMD Bass/Tile program. Returns compiled Bacc module."""
    nc = bacc.Bacc("TRN2", target_bir_lowering=False, debug=False)

    def inp(name, shape, dt):
        return nc.dram_tensor(name, list(shape), dt, kind="ExternalInput").ap()

    d_pclT = inp("pclT", (4, NB), f16)
    d_delta0 = inp("delta0", (4, NB), f16)
    d_Wf1 = inp("Wf1", (3, F), f16)
    d_bf1 = inp("bf1", (F, 1), f32)
    d_WfW = inp("WfW", (F, F), f16)
    d_bg = inp("bg", (F, 1), f32)
    d_W0g = inp("W0g", (3, F), f16)
    d_W0gn = inp("W0gn", (3, F), f16)
    d_I128 = inp("I128", (F, F), f16)
    d_Wb1 = inp("Wb1", (F, F), f16)
    d_Wb2 = inp("Wb2", (F, F), f16)
    d_pb1 = inp("pb1", (F, 1), f32)     # +b1
    d_nb1 = inp("nb1", (F, 1), f32)     # -b1
    d_pb2 = inp("pb2", (F, 1), f32)     # +b2eff
    d_nb2 = inp("nb2", (F, 1), f32)     # -b2eff
    d_WoS = inp("WoS", (F, 3 * STEPS), f16)
    d_bo4 = inp("bo4", (4, STEPS), f32)        # 4*bo_eff per step
    d_cboL = inp("cboL", (4, 2 * STEPS), f32)  # edge count corrections
    d_cboR = inp("cboR", (4, 2 * STEPS), f32)
    d_eL = inp("eL", (F, 3 * STEPS), f16)
    d_eLn = inp("eLn", (F, 3 * STEPS), f16)
    d_eR = inp("eR", (F, 3 * STEPS), f16)
    d_eRn = inp("eRn", (F, 3 * STEPS), f16)
    d_flagL = inp("flagL", (4, 1), f32)
    d_flagR = inp("flagR", (4, 1), f32)
    d_out = nc.dram_tensor("outT", [4, CHUNK], f16, kind="ExternalOutput").ap()

    from contextlib import ExitStack
    with tile.TileContext(nc) as tc, ExitStack() as ctx:
        cpool = ctx.enter_context(tc.tile_pool(name="const", bufs=1))
        hpool = ctx.enter_context(tc.tile_pool(name="h", bufs=4))
        tp2 = ctx.enter_context(tc.tile_pool(name="t2", bufs=6))
        tpool = ctx.enter_context(tc.tile_pool(name="tiny", bufs=2))
        psp = ctx.enter_context(tc.tile_pool(name="ps", bufs=3, space="PSUM"))
        psbc = ctx.enter_context(tc.tile_pool(name="psBC", bufs=2, space="PSUM"))
        pssc = ctx.enter_context(tc.tile_pool(name="psS", bufs=1, space="PSUM"))
        h0pool = ctx.enter_context(tc.tile_pool(name="h0p", bufs=12))
        h2pool = ctx.enter_context(tc.tile_pool(name="h2p", bufs=8))

        def load(dram, shape, dt, tag):
            t = cpool.tile(list(shape), dt, tag=tag)
            nc.sync.dma_start(t[:], dram[:])
            return t

        # load order = first-use order: the preamble's first matmuls need
        # pclT+W0g+Wf1 — don't queue them behind the delta buffers
        pclT = load(d_pclT, (4, NB), f16, "pclT")
        W0g = load(d_W0g, (3, F), f16, "W0g")
        Wf1 = load(d_Wf1, (3, F), f16, "Wf1")
        bf1 = load(d_bf1, (F, 1), f32, "bf1")
        WfW = load(d_WfW, (F, F), f16, "WfW")
        bg = load(d_bg, (F, 1), f32, "bg")
        W0gn = load(d_W0gn, (3, F), f16, "W0gn")
        I128 = load(d_I128, (F, F), f16, "I128")
        delta_a = load(d_delta0, (4, NB), f16, "delta_a")
        delta_b = load(d_delta0, (4, NB), f16, "delta_b")
        Wb1 = load(d_Wb1, (F, F), f16, "Wb1")
        Wb2 = load(d_Wb2, (F, F), f16, "Wb2")
        pb1 = load(d_pb1, (F, 1), f32, "pb1")
        nb1 = load(d_nb1, (F, 1), f32, "nb1")
        pb2 = load(d_pb2, (F, 1), f32, "pb2")
        nb2 = load(d_nb2, (F, 1), f32, "nb2")
        WoS = load(d_WoS, (F, 3 * STEPS), f16, "WoS")
        bo4 = load(d_bo4, (4, STEPS), f32, "bo4")
        cboL = load(d_cboL, (4, 2 * STEPS), f32, "cboL")
        cboR = load(d_cboR, (4, 2 * STEPS), f32, "cboR")
        eL = load(d_eL, (F, 3 * STEPS), f16, "eL")
        eLn = load(d_eLn, (F, 3 * STEPS), f16, "eLn")
        eR = load(d_eR, (F, 3 * STEPS), f16, "eR")
        eRn = load(d_eRn, (F, 3 * STEPS), f16, "eRn")
        flagL = load(d_flagL, (4, 1), f32, "flagL")
        flagR = load(d_flagR, (4, 1), f32, "flagR")

        Gk = cpool.tile([F, R4], f16, tag="Gk")
        h3_a = cpool.tile([F, R4], f16, tag="h3_a")
        h3_b = cpool.tile([F, R4], f16, tag="h3_b")
        A0e = cpool.tile([F, NB], f16, tag="A0e")
        G0 = cpool.tile([F, NP], f16, tag="G0")

        # scatter PSUM: one bank holding 3 rotating [3,512] slots at
        # partition offsets 0/32/64 (96 is not a legal base partition)
        ps_sc = pssc.tile([F, 512], f32, tag="S")
        sc_ctr = [0]
        rs_ctr = [0]

        def pairv(t, fd):
            # [P, 1024] tile -> strided pair view [P, 2, fd] (halves at
            # byte offsets 0 and 512 cols)
            return t[:].rearrange("p (h n) -> p h n", h=2)[:, :, 0:fd]

        # ---- greedy tri-engine balancer for elementwise work ----
        # constants calibrated against the TimelineSim instruction model
        load_ns = {"ACT": 0.0, "DVE": 0.0, "POOL": 0.0, "DMA": 0.0}

        def c_act(fd):
            return fd * 0.833 + 190

        def c_dve_ps(fd):
            return fd * 1.042 + 125

        def c_dve_stt_sb(fd):
            return fd * 1.042 + 60

        def c_dve_tt(fd):
            return fd * 0.521 + 60

        def c_pool_tt(fd):
            return fd * 1.984 + 120

        def pick(options):
            # options: list of (fn, [(eng, cost), ...])
            best = None
            for fn, usages in options:
                new = dict(load_ns)
                for e, c in usages:
                    new[e] += c
                key = (max(new.values()), sum(c for _, c in usages))
                if best is None or key < best[0]:
                    best = (key, fn, usages)
            for e, c in best[2]:
                load_ns[e] += c
            best[1]()

        def charge(eng, cost):
            load_ns[eng] += cost

        def relu_op(dst, src, fd, bias=None):
            # psum -> sbuf relu, optional per-partition bias. GPSIMD cannot
            # read PSUM, but a DMA drain (psum -> sbuf staging) lets Pool do
            # the relu as TT-max against zeros (bias-free case only).
            def on_act():
                nc.scalar.activation(dst, src, AF.Relu,
                                     bias=(bias[:, :] if bias is not None else 0.0))
            def on_dve():
                if bias is not None:
                    nc.vector.tensor_scalar(dst, src, bias[:, :], 0.0, ALU.add, ALU.max)
                else:
                    nc.vector.tensor_scalar_max(dst, src, 0.0)
            pick([(on_act, [("ACT", c_act(fd))]),
                  (on_dve, [("DVE", c_dve_ps(fd))])])

        def copy_op(dst, src, fd):
            def on_act():
                nc.scalar.activation(dst, src, AF.Copy)
            def on_dve():
                nc.vector.tensor_copy(dst, src)
            pick([(on_act, [("ACT", c_act(fd))]),
                  (on_dve, [("DVE", c_dve_ps(fd))])])

        def fused_resid_pair(dst_pair, ps_pair, hin_pair, fd, pb, nb):
            # dst = max(ps, -b) + hin over a k-pair (== hin + relu(ps+b) - b)
            # GPSIMD cannot read PSUM and only supports TensorTensor-class
            # ops: its 2-op path (ACT relu -> sbuf, Pool TT add) is exact
            # only when b == 0, so it is only emitted in zero-bias programs.
            fd2 = 2 * fd
            def on_dve():
                nc.vector.scalar_tensor_tensor(dst_pair, ps_pair, nb[:, :],
                                               hin_pair, ALU.max, ALU.add)
            def on_2op():
                t = tp2.tile([F, 1024], f16, tag="t2")
                tv = pairv(t, fd)
                nc.scalar.activation(tv, ps_pair, AF.Relu, bias=pb[:, :])
                if zb:
                    nc.vector.tensor_add(dst_pair, tv, hin_pair)
                else:
                    nc.vector.scalar_tensor_tensor(dst_pair, tv, nb[:, :],
                                                   hin_pair, ALU.add, ALU.add)
            def on_2op_pool():
                t = tp2.tile([F, 1024], f16, tag="t2")
                tv = pairv(t, fd)
                nc.scalar.activation(tv, ps_pair, AF.Relu)
                nc.gpsimd.tensor_add(dst_pair, tv, hin_pair)
            opts = [(on_dve, [("DVE", c_dve_ps(fd2))]),
                    (on_2op, [("ACT", c_act(fd2)),
                              ("DVE", c_dve_tt(fd2) if zb else c_dve_stt_sb(fd2))])]
            if zb:
                opts.append(
                    (on_2op_pool, [("ACT", c_act(fd2)), ("POOL", c_pool_tt(fd2))]))
            pick(opts)

        def delta_update(dst, ps, din, fd, step):
            # dst = ps + 4*bo_eff[step] + din
            sc = bo4[0:3, step:step + 1]
            def on_dve():
                nc.vector.scalar_tensor_tensor(dst, ps, sc, din, ALU.add, ALU.add)
            def on_2op():
                t = hpool.tile([4, 512], f16, tag="t4")
                nc.scalar.activation(t[0:3, :fd], ps, AF.Identity, bias=sc)
                nc.vector.tensor_add(dst, t[0:3, :fd], din)
            def on_2op_pool():
                t = hpool.tile([4, 512], f16, tag="t4")
                nc.scalar.activation(t[0:3, :fd], ps, AF.Identity, bias=sc)
                nc.gpsimd.tensor_add(dst, t[0:3, :fd], din)
            pick([(on_dve, [("DVE", c_dve_ps(fd))]),
                  (on_2op, [("ACT", c_act(fd)), ("DVE", c_dve_tt(fd))]),
                  (on_2op_pool, [("ACT", c_act(fd)), ("POOL", c_pool_tt(fd))])])

        # one column at the k=2/k=3 boundary is read (as cone garbage) by the
        # interleaved scatter before any tile writes it on step 0
        nc.vector.memset(h3_a[:, 3 * NP - 1:3 * NP], 0.0)
        nc.vector.memset(h3_b[:, 3 * NP - 1:3 * NP], 0.0)

        # ---------------- preamble: A0e, G0, Gk ----------------
        # per-tile interleave so step-0's early tiles unblock fast; Gk is
        # built with ONE 4-block strided TT per tile (the four k-shifts are
        # consecutive A0e columns, G0 broadcasts along the k dim)
        GkvF = Gk[:].rearrange("p (k n) -> p k n", k=K)

        def gk_quad(ci):
            c0, fd = _CH512[ci]
            dstq = GkvF[:, :, c0:c0 + fd]
            g0b = G0[:, c0:c0 + fd].unsqueeze(1).broadcast_to([F, K, fd])
            a0b = A0e[:, GW + OFF[0] + c0:GW + OFF[0] + c0 + fd]
            a0q = bass.AP(a0b.tensor, a0b.offset,
                          [list(a0b.ap[0]), [1, K], [1, fd]])
            def on_dve():
                nc.vector.tensor_add(dstq, g0b, a0q)
            def on_pool():
                nc.gpsimd.tensor_add(dstq, g0b, a0q)
            if ci < 2:
                on_dve()
                charge("DVE", c_dve_tt(K * fd))
            else:
                pick([(on_dve, [("DVE", c_dve_tt(K * fd))]),
                      (on_pool, [("POOL", c_pool_tt(K * fd))])])

        for ci in range(len(_CHNB) + 1):
            if ci < len(_CHNB):
                c0, fd = _CHNB[ci]
                ps = psp.tile([F, 512], f32, tag="ps")
                nc.tensor.matmul(ps[:, :fd], W0g[:, :], pclT[0:3, c0:c0 + fd],
                                 start=True, stop=True)
                copy_op(A0e[:, c0:c0 + fd], ps[:, :fd], fd)
            if ci < len(_CH512):
                c0, fd = _CH512[ci]
                ps = psp.tile([F, 512], f32, tag="ps")
                nc.tensor.matmul(ps[:, :fd], Wf1[:, :],
                                 pclT[0:3, GW + c0:GW + c0 + fd],
                                 start=True, stop=True)
                hf = hpool.tile([F, 512], f16, tag="h0")
                nc.scalar.activation(hf[:, :fd], ps[:, :fd], AF.Relu, bias=bf1[:, :])
                ps2 = psp.tile([F, 512], f32, tag="ps")
                nc.tensor.matmul(ps2[:, :fd], WfW[:, :], hf[:, :fd],
                                 start=True, stop=False)
                nc.tensor.matmul(ps2[:, :fd], W0gn[:, :],
                                 pclT[0:3, GW + c0:GW + c0 + fd],
                                 start=False, stop=True)
                nc.scalar.activation(G0[:, c0:c0 + fd], ps2[:, :fd], AF.Identity,
                                     bias=bg[:, :])
            if 0 <= ci - 1 < len(_CH512):
                gk_quad(ci - 1)

        # ---------------- langevin steps ----------------
        def emit_rep(final_rep):
            for step in range(STEPS):
                d_in = delta_a if step % 2 == 0 else delta_b
                d_out_t = delta_b if step % 2 == 0 else delta_a
                h3 = h3_a if step % 2 == 0 else h3_b
                h3kv = h3[:].rearrange("p (k n) -> p k n", k=K)
                final = (step == STEPS - 1) and final_rep
                s3 = slice(3 * step, 3 * step + 3)

                def emit_passA(cb, kks=(0, 2)):
                    c0, fd = _CH512[cb]
                    for kk in kks:
                        h0p = h0pool.tile([F, 1024], f16, tag="h0")
                        for j in range(2):
                            k = kk + j
                            ps = psp.tile([F, 512], f32, tag="ps")
                            nc.tensor.matmul(ps[:, :fd], I128[:, :],
                                             Gk[:, k * NP + c0:k * NP + c0 + fd],
                                             start=True, stop=False)
                            nc.tensor.matmul(
                                ps[:, :fd], W0g[:, :],
                                d_in[0:3, GW + OFF[k] + c0:GW + OFF[k] + c0 + fd],
                                start=False, stop=True)
                            relu_op(h0p[:, 512 * j:512 * j + fd], ps[:, :fd], fd)
                        h0s[(kk, cb)] = h0p

                def emit_passB(cb, kks=(0, 2)):
                    c0, fd = _CH512[cb]
                    for kk in kks:
                        h0p = h0s[(kk, cb)]
                        ps = psbc.tile([F, 1024], f32, tag="bc")
                        for j in range(2):
                            nc.tensor.matmul(ps[:, 512 * j:512 * j + fd], Wb1[:, :],
                                             h0p[:, 512 * j:512 * j + fd],
                                             start=True, stop=True)
                        h2p = h2pool.tile([F, 1024], f16, tag="h2")
                        fused_resid_pair(pairv(h2p, fd), pairv(ps, fd),
                                         pairv(h0p, fd), fd, pb1, nb1)
                        h2s[(kk, cb)] = h2p

                def emit_passC(cb, kks=(0, 2)):
                    c0, fd = _CH512[cb]
                    for kk in kks:
                        h2p = h2s[(kk, cb)]
                        ps = psbc.tile([F, 1024], f32, tag="bc")
                        for j in range(2):
                            nc.tensor.matmul(ps[:, 512 * j:512 * j + fd], Wb2[:, :],
                                             h2p[:, 512 * j:512 * j + fd],
                                             start=True, stop=True)
                        fused_resid_pair(h3kv[:, kk:kk + 2, c0:c0 + fd],
                                         pairv(ps, fd), pairv(h2p, fd), fd, pb2, nb2)

                def mirror_fix(flag, src_l, dst_l0, w):
                    # mirror guards at global edges (flag=0 -> no-op on
                    # interior); handles w contiguous dst columns in one shot
                    # with a broadcast source
                    dst = d_out_t[0:3, GW + dst_l0:GW + dst_l0 + w]
                    src = d_out_t[0:3, GW + src_l:GW + src_l + 1].broadcast_to([3, w])
                    t = tpool.tile([4, 2], f16, tag="mir")
                    nc.vector.tensor_sub(t[0:3, :w], src, dst)
                    nc.vector.tensor_scalar_mul(t[0:3, :w], t[0:3, :w], flag[0:3, :])
                    nc.vector.tensor_add(dst, dst, t[0:3, :w])

                def emit_scatter(cb):
                    c0, fd = _CH512[cb]
                    p0 = 32 * (sc_ctr[0] % 3)
                    sc_ctr[0] += 1
                    ps = ps_sc[p0:p0 + 4, :]
                    mms = []
                    for k in range(K):
                        st = k * NP + c0 - OFF[k]
                        mms.append((ps[0:3, :fd], WoS[:, s3], h3[:, st:st + fd]))
                    if cb == 0:
                        pcol = ps[0:3, HALO:HALO + 1]
                        for col in (HALO, HALO + 1, NP + HALO):
                            mms.append((pcol, eL[:, s3], h3[:, col:col + 1]))
                        mms.append((pcol, eLn[:, s3],
                                    h3[:, 3 * NP + HALO - 1:3 * NP + HALO]))
                    if cb == len(_CH512) - 1:
                        lN = HALO + CHUNK - 1
                        pN = ps[0:3, lN - c0:lN - c0 + 1]
                        mms.append((pN, eR[:, s3], h3[:, 3 * NP + lN:3 * NP + lN + 1]))
                        for col in (lN + 2, NP + lN + 1):
                            mms.append((pN, eRn[:, s3], h3[:, col:col + 1]))
                        mms.append((ps[0:3, lN - 1 - c0:lN - c0], eRn[:, s3],
                                    h3[:, lN + 1:lN + 2]))
                    for i, (o, w, m) in enumerate(mms):
                        nc.tensor.matmul(o, w, m, start=(i == 0),
                                         stop=(i == len(mms) - 1))
                    delta_update(d_out_t[0:3, GW + c0:GW + c0 + fd], ps[0:3, :fd],
                                 d_in[0:3, GW + c0:GW + c0 + fd], fd, step)
                    if cb == 0:
                        nc.vector.tensor_add(
                            d_out_t[0:3, GW + HALO:GW + HALO + 2],
                            d_out_t[0:3, GW + HALO:GW + HALO + 2],
                            cboL[0:3, 2 * step:2 * step + 2])
                        charge("DVE", 80)
                    if cb == len(_CH512) - 1:
                        r0 = HALO + CHUNK - 2
                        nc.vector.tensor_add(
                            d_out_t[0:3, GW + r0:GW + r0 + 2],
                            d_out_t[0:3, GW + r0:GW + r0 + 2],
                            cboR[0:3, 2 * step:2 * step + 2])
                        charge("DVE", 80)
                    if final:
                        # stream the finished output tile to DRAM now instead
                        # of one big DMA after the last tile
                        a = max(0, c0 - HALO)
                        b = min(CHUNK, c0 + fd - HALO)
                        if b > a:
                            nc.sync.dma_start(
                                d_out[:, a:b],
                                d_out_t[0:4, GW + HALO + a:GW + HALO + b])

                h0s = {}
                h2s = {}
                nblk = len(_CH512)
                for cb in range(nblk + 4):
                    for kks in ((0,), (2,)):
                        if cb < nblk:
                            emit_passA(cb, kks)
                        if 0 <= cb - 3 < nblk:
                            emit_passC(cb - 3, kks)
                        if 0 <= cb - 1 < nblk:
                            emit_passB(cb - 1, kks)
                    if 0 <= cb - 4 < nblk:
                        emit_scatter(cb - 4)

                if not final:
                    mirror_fix(flagL, HALO, HALO - 2, 2)
                    mirror_fix(flagR, HALO + CHUNK - 1, HALO + CHUNK, 1)

        if loop_n:
            with tc.For_i(0, loop_n, 1):
                emit_rep(False)
            emit_rep(True)
        else:
            for rep in range(reps):
                emit_rep(rep == reps - 1)

    nc.compile()
    return nc


def host_prep(inputs):
    """Slice/transpose/pad inputs per core; build weight-variant constants."""
    pcl = np.asarray(inputs["pcl_noisy"], np.float32)
    Wf1 = np.asarray(inputs["Wf1"], np.float32)
    bf1 = np.asarray(inputs["bf1"], np.float32)
    Wf2 = np.asarray(inputs["Wf2"], np.float32)
    bf2 = np.asarray(inputs["bf2"], np.float32)
    W0 = np.asarray(inputs["W0"], np.float32)
    b0 = np.asarray(inputs["b0"], np.float32)
    Wb = np.asarray(inputs["Wb"], np.float32)
    bb = np.asarray(inputs["bb"], np.float32)
    Wo = np.asarray(inputs["Wo"], np.float32)
    bo = np.asarray(inputs["bo"], np.float32)

    W0g = W0[:3]
    WfW = Wf2 @ W0[3:]
    bg = bf2 @ W0[3:] + b0
    offs = np.arange(-(K - 1) // 2, (K - 1) // 2 + 1)
    nbr = np.clip(np.arange(N)[:, None] + offs, 0, N - 1).reshape(-1)
    counts = np.bincount(nbr, minlength=N).astype(np.float32)

    b1 = bb[0]
    b2eff = bb[1] + Wb[1].T @ b1
    bsum = b1 + b2eff
    svals = [S0 * DECAY ** i for i in range(STEPS)]
    WoS = np.concatenate([s * Wo for s in svals], axis=1)          # [128, 12]
    bo_eff = np.stack([s * (Wo.T @ bsum + bo) for s in svals], 1)  # [3, STEPS]
    bo4 = np.zeros((4, STEPS), np.float32)
    bo4[0:3] = 4.0 * bo_eff

    hf = np.float16
    shared = {
        "Wf1": Wf1.astype(hf), "bf1": bf1.reshape(F, 1),
        "WfW": WfW.astype(hf), "bg": bg.reshape(F, 1),
        "W0g": W0g.astype(hf), "W0gn": (-W0g).astype(hf),
        "I128": np.eye(F, dtype=np.float32).astype(hf),
        "Wb1": Wb[0].astype(hf), "Wb2": Wb[1].astype(hf),
        "pb1": b1.reshape(F, 1).astype(np.float32),
        "nb1": (-b1).reshape(F, 1).astype(np.float32),
        "pb2": b2eff.reshape(F, 1).astype(np.float32),
        "nb2": (-b2eff).reshape(F, 1).astype(np.float32),
        "WoS": WoS.astype(hf),
        "bo4": bo4,
    }
    zeros_e = np.zeros((F, 3 * STEPS), np.float16)
    zeros_c = np.zeros((4, 2 * STEPS), np.float32)
    # edge-count corrections: counts - 4 is nonzero only at global cols
    # {0, 1} (left) and {N-2, N-1} (right)
    cboL = np.zeros((4, 2 * STEPS), np.float32)
    cboR = np.zeros((4, 2 * STEPS), np.float32)
    for s in range(STEPS):
        cboL[0:3, 2 * s:2 * s + 2] = (counts[0:2] - 4.0) * bo_eff[:, s:s + 1]
        cboR[0:3, 2 * s:2 * s + 2] = (counts[N - 2:N] - 4.0) * bo_eff[:, s:s + 1]

    in_maps = []
    for core in range(N_CORES):
        b, ch = core // 4, core % 4
        g0 = ch * CHUNK - HALO
        idx = np.clip(np.arange(g0 - GW, g0 + NP + GW), 0, N - 1)
        pclT = np.empty((4, NB), np.float16)
        pclT[0:3] = pcl[b, idx].T.astype(np.float16)
        pclT[3] = 0.0
        delta0 = np.zeros((4, NB), np.float16)
        isL, isR = ch == 0, ch == 3
        m = dict(shared)
        m["pclT"] = pclT
        m["delta0"] = delta0
        m["eL"] = (WoS.astype(hf) if isL else zeros_e)
        m["eLn"] = ((-WoS).astype(hf) if isL else zeros_e)
        m["eR"] = (WoS.astype(hf) if isR else zeros_e)
        m["eRn"] = ((-WoS).astype(hf) if isR else zeros_e)
        m["cboL"] = (cboL if isL else zeros_c)
        m["cboR"] = (cboR if isR else zeros_c)
        m["flagL"] = np.full((4, 1), 1.0 if isL else 0.0, np.float32)
        m["flagR"] = np.full((4, 1), 1.0 if isR else 0.0, np.float32)
        in_maps.append(m)
    return in_maps


_CACHED = {}


def _get_program(reps=1, zb=True):
    key = (reps, zb)
    if key not in _CACHED:
        _CACHED[key] = build_program(reps, zb=zb)
    return _CACHED[key]


def kernel(**inputs):
    zb = all(
        not np.any(np.asarray(inputs[k]))
        for k in ("bf1", "bf2", "b0", "bb", "bo"))
    nc = _get_program(1, zb)
    in_maps = host_prep(inputs)
    res = run_bass_kernel_spmd(nc, in_maps, list(range(N_CORES)))
    pcl = np.asarray(inputs["pcl_noisy"], np.float32)
    out = np.empty((B, N, D), np.float32)
    for core in range(N_CORES):
        b, ch = core // 4, core % 4
        sl = slice(ch * CHUNK, (ch + 1) * CHUNK)
        out[b, sl] = pcl[b, sl] + res.results[core]["outT"][0:3].T.astype(np.float32)
    return out
